# revision 1
# baseline (speedup 1.0000x reference)
"""Trainium2 Bass kernel for nn_EventMemoryCell (B=4096, D=H=512, S=16).

Strategy (hardcoded for the spec shapes):
  - Data parallel over batch across 8 cores (512 rows each), parameters
    replicated; one SPMD NEFF.
  - Everything on-device runs in a transposed (feature-on-partition,
    batch-on-free) layout, so every matmul contracts over partitions and
    the LSTM recurrence needs no transposes.
  - mem_seq is never materialized: for s<15,
      xg[s] = A@slots_old[s+1] + C@cum_old[s+1] + d*(delta_old[s+1]+1)
              + shared,  shared = A@(leak*x) + 2*C@x + (b_ih+b_hh)
    and for s=15 xg[15] = A@new_slot + shared (cum part == 2x is in shared,
    delta part == 0).
  - Attention: sims = slots . ((Wk^T Wq) x), so keys (B,S,H) is never built.
  - dtypes: slots/cum streams + A/C in bf16; recurrence (W_hh,h), prologue
    attention mats and epilogue Wo in float32r (tf32-like); shared kept in
    fp32 and added into PSUM with the vector engine.
"""
import sys

sys.path.insert(0, "/opt/trn_rl_repo")

import numpy as np
import ml_dtypes

import concourse.bass as bass
import concourse.tile as tile
import concourse.mybir as mybir
from concourse import bacc, bass_utils

F32 = mybir.dt.float32
F32R = mybir.dt.float32r
F16 = mybir.dt.float16
BF16 = mybir.dt.float16  # fp16: same speed as bf16, 3 more mantissa bits
AF = mybir.ActivationFunctionType
BF = np.float16

B, S, D, H = 4096, 16, 512, 512
NCORES = 8
BC = B // NCORES            # 512 batch rows per core
G4 = 4 * H                  # 2048 gate rows
KD = D // 128               # 4 k-tiles over D/H
KM = G4 // 128              # 16 gate partition tiles
KO = (2 * H + D) // 128     # 12 k-tiles for the output projection

# interleaved gate-tile order [0,4,8,12, 1,5,9,13, ...]: finish chunk j's
# i/f/g/o gates together so c/h updates start early
M_ORDER = [j + 4 * i for j in range(4) for i in range(4)]

_BUILT = None
DEBUG_STEPS = ()  # set before first kernel() call to dump c/h after these steps


def _build_program():
    nc = bacc.Bacc("TRN2", target_bir_lowering=False, debug=False)

    def din(name, shape, dt):
        return nc.dram_tensor(name, list(shape), dt, kind="ExternalInput").ap()

    xT = din("xT", (D, BC), F32R)
    s0T = din("s0T", (D, BC), BF16)
    slotsT = din("slotsT", (S - 1, D, BC), BF16)
    cumT = din("cumT", (S - 1, D, BC), BF16)
    d1T = din("d1T", (1, (S - 1) * BC), BF16)
    hpT = din("hpT", (H, BC), F16)
    m2T = din("m2T", (D, D), F32R)
    wvT = din("wvT", (D, D), F32R)
    wrT = din("wrT", (D, 1), F32R)
    waT = din("waT", (D, 1), F32R)
    bv = din("bv", (D, 1), F32)
    br = din("br", (1, 1), F32)
    ba = din("ba", (1, 1), F32)
    bo = din("bo", (H, 1), F32)
    aT = din("aT", (D, G4), BF16)
    cT = din("cT", (D, G4), BF16)
    d_row = din("d_row", (1, G4), BF16)
    bias_row = din("bias_row", (1, G4), BF16)
    whT = din("whT", (H, G4), F16)
    woT = din("woT", (2 * H + D, H), F16)
    d_col = din("d_col", (G4, 1), F32)
    hnT = nc.dram_tensor("hnT", [H, BC], F32, kind="ExternalOutput").ap()
    dbg = {}
    for ds in DEBUG_STEPS:
        dbg[ds] = (nc.dram_tensor(f"cD{ds}", [H, BC], F32, kind="ExternalOutput").ap(),
                   nc.dram_tensor(f"hD{ds}", [H, BC], F32, kind="ExternalOutput").ap())

    r3 = lambda ap: ap.rearrange("(kt p) b -> p kt b", p=128)
    r2 = lambda ap: ap.rearrange("(kt p) o -> p (kt o)", p=128)

    with tile.TileContext(nc) as tc:
        wp = tc.alloc_tile_pool(name="wp", bufs=1)
        st_p = tc.alloc_tile_pool(name="state", bufs=1)
        pp = tc.alloc_tile_pool(name="pp", bufs=8, space="PSUM")

        # ---- resident weights / constants ----
        aT_sb = wp.tile([128, KD, G4], BF16, name="aT_sb")
        cT_sb = wp.tile([128, KD, G4], BF16, name="cT_sb")
        whT_sb = wp.tile([128, KD, G4], F16, name="whT_sb")
        dr_sb = wp.tile([1, G4], BF16, name="dr_sb")
        nc.sync.dma_start(dr_sb[:], d_row)
        bias_sb = wp.tile([1, G4], BF16, name="bias_sb")
        nc.sync.dma_start(bias_sb[:], bias_row)
        d1_sb = wp.tile([1, (S - 1) * BC], BF16, name="d1_sb")
        nc.sync.dma_start(d1_sb[:], d1T)
        wo_sb = wp.tile([128, KO, H], F16, name="wo_sb")
        d_sb = wp.tile([128, KM], F32, name="d_sb")
        nc.sync.dma_start(d_sb[:], r2(d_col))
        bo_sb = wp.tile([128, KD], F32, name="bo_sb")
        nc.sync.dma_start(bo_sb[:], r2(bo))
        ones_bf = wp.tile([1, BC], BF16, name="ones_bf")
        nc.vector.memset(ones_bf[:], 1.0)
        ones_col = wp.tile([128, 1], BF16, name="ones_col")
        nc.vector.memset(ones_col[:], 1.0)

        xt = st_p.tile([128, KD, BC], F32R, name="xt")
        nc.sync.dma_start(xt[:], r3(xT))
        shared_sb = st_p.tile([128, KM, BC], F32, name="shared_sb")
        ut = st_p.tile([128, KD, BC], BF16, name="ut")
        P_t = st_p.tile([128, KD, BC], BF16, name="P_t")
        c_t = [st_p.tile([128, BC], F32, name=f"c{k}", tag=f"c{k}") for k in range(KD)]
        h_t = [[st_p.tile([128, BC], F16, name=f"h{pq}_{k}", tag=f"h{pq}_{k}")
                for k in range(KD)] for pq in range(2)]
        g_row = st_p.tile([1, BC], BF16, name="g_row")
        max_row = st_p.tile([1, BC], F32, name="max_row")

        MSIG, MTANH = AF.Sigmoid, AF.Tanh

        def sims_row(idx, tsrc, tagsfx, tpool):
            # running max over slots: max_row = max(max_row, slots_s . u)
            srp = pp.tile([128, BC], F32, name=f"srp{tagsfx}", tag="ps")
            mm_group(srp[0:1, :], [(ones_col[:], tsrc[:, k, :]) for k in range(KD)])
            if idx == 0:
                nc.scalar.activation(max_row[:], srp[0:1, :], AF.Copy)
            else:
                srow = tpool.tile([1, BC], F32, name=f"sr{tagsfx}", tag="srow",
                                  bufs=1)
                nc.scalar.activation(srow[:], srp[0:1, :], AF.Copy)
                nc.vector.tensor_max(max_row[:], max_row[:], srow[:])

        def mm_group(ps_ap, terms):
            n = len(terms)
            for i, (lh, rh) in enumerate(terms):
                nc.tensor.matmul(ps_ap, lh, rh, start=(i == 0), stop=(i == n - 1))

        # ================= prologue =================
        with tc.tile_pool(name="prop", bufs=1) as prop:
            m2_sb = prop.tile([128, KD, D], F32R, name="m2_sb")
            nc.sync.dma_start(m2_sb[:], r3(m2T))
            wv_sb = prop.tile([128, KD, D], F32R, name="wv_sb")
            nc.sync.dma_start(wv_sb[:], r3(wvT))
            wr_sb = prop.tile([128, KD], F32R, name="wr_sb")
            nc.sync.dma_start(wr_sb[:], r2(wrT))
            wa_sb = prop.tile([128, KD], F32R, name="wa_sb")
            nc.sync.dma_start(wa_sb[:], r2(waT))
            bv_sb = prop.tile([128, KD], F32, name="bv_sb")
            nc.sync.dma_start(bv_sb[:], r2(bv))
            br_sb = prop.tile([1, 1], F32, name="br_sb")
            nc.sync.dma_start(br_sb[:], br)
            ba_sb = prop.tile([1, 1], F32, name="ba_sb")
            nc.sync.dma_start(ba_sb[:], ba)
            s0_sb = prop.tile([128, KD, BC], BF16, name="s0_sb")
            nc.sync.dma_start(s0_sb[:], r3(s0T))
            # big resident weights: emitted after the prologue-critical DMAs
            nc.sync.dma_start(aT_sb[:], r3(aT))
            nc.sync.dma_start(cT_sb[:], r3(cT))
            nc.sync.dma_start(whT_sb[:], r3(whT))
            vt = prop.tile([128, KD, BC], BF16, name="vt")
            lx = prop.tile([128, KD, BC], BF16, name="lx")
            xbf = prop.tile([128, KD, BC], BF16, name="xbf")
            x2bf = prop.tile([128, KD, BC], BF16, name="x2bf")
            r_row = prop.tile([1, BC], BF16, name="r_row")
            lk_row = prop.tile([1, BC], BF16, name="lk_row")
            R_bc = prop.tile([128, BC], BF16, name="R_bc")
            L_bc = prop.tile([128, BC], BF16, name="L_bc")

            nc.scalar.activation(xbf[:], xt[:].bitcast(F32), AF.Copy)
            nc.scalar.activation(x2bf[:], xt[:].bitcast(F32), AF.Copy, scale=2.0)

            # u = (Wk^T Wq) x ; v = Wv x + bv   (feature-major)
            for m in range(KD):
                ups = pp.tile([128, BC], F32, name=f"ups{m}", tag="ps")
                mm_group(ups[:], [(m2_sb[:, k, 128 * m:128 * (m + 1)], xt[:, k, :])
                                  for k in range(KD)])
                nc.scalar.activation(ut[:, m, :], ups[:], AF.Copy)
            for m in range(KD):
                vps = pp.tile([128, BC], F32, name=f"vps{m}", tag="ps")
                mm_group(vps[:], [(wv_sb[:, k, 128 * m:128 * (m + 1)], xt[:, k, :])
                                  for k in range(KD)])
                nc.scalar.activation(vt[:, m, :], vps[:], AF.Identity,
                                     bias=bv_sb[:, m:m + 1])

            # r / leak rows
            rps = pp.tile([128, BC], F32, name="rps", tag="ps")
            mm_group(rps[0:1, :], [(wr_sb[:, k:k + 1], xt[:, k, :]) for k in range(KD)])
            nc.scalar.activation(r_row[:], rps[0:1, :], MSIG, bias=br_sb[0:1, 0:1])
            lps = pp.tile([128, BC], F32, name="lps", tag="ps")
            mm_group(lps[0:1, :], [(wa_sb[:, k:k + 1], xt[:, k, :]) for k in range(KD)])
            nc.scalar.activation(lk_row[:], lps[0:1, :], MSIG, bias=ba_sb[0:1, 0:1])

            # broadcast r/leak rows to 128 partitions via a K=1 matmul
            bps = pp.tile([128, BC], F32, name="bps", tag="ps")
            mm_group(bps[:], [(ones_bf[0:1, 0:128], r_row[:])])
            nc.scalar.activation(R_bc[:], bps[:], AF.Copy)
            bps2 = pp.tile([128, BC], F32, name="bps2", tag="ps")
            mm_group(bps2[:], [(ones_bf[0:1, 0:128], lk_row[:])])
            nc.scalar.activation(L_bc[:], bps2[:], AF.Copy)

            # P = r*slots0 + (1-r)*v = v + r*(slots0 - v);  lx = leak*x
            for k in range(KD):
                t1 = prop.tile([128, BC], BF16, name=f"pt{k}", tag="ptmp", bufs=2)
                nc.vector.tensor_sub(t1[:], s0_sb[:, k, :], vt[:, k, :])
                nc.vector.tensor_mul(t1[:], R_bc[:], t1[:])
                nc.vector.tensor_add(P_t[:, k, :], vt[:, k, :], t1[:])
                nc.vector.tensor_mul(lx[:, k, :], L_bc[:], xbf[:, k, :])

            # sims row 0 (original slot 0)
            ts0 = prop.tile([128, KD, BC], BF16, name="ts0")
            nc.vector.tensor_mul(ts0[:], s0_sb[:], ut[:])
            sims_row(0, ts0, "p", prop)

            # shared = A@lx + C@(2x) + (b_ih+b_hh) x ones
            for m in range(KM):
                sl = slice(128 * m, 128 * (m + 1))
                sps = pp.tile([128, BC], F32, name=f"sps{m}", tag="ps")
                terms = [(aT_sb[:, k, sl], lx[:, k, :]) for k in range(KD)]
                terms += [(cT_sb[:, k, sl], x2bf[:, k, :]) for k in range(KD)]
                terms += [(bias_sb[0:1, sl], ones_bf[:])]
                mm_group(sps[:], terms)
                nc.scalar.activation(shared_sb[:, m, :], sps[:], AF.Copy)

        # ================= LSTM over S steps =================
        with tc.tile_pool(name="sp", bufs=2) as sp, \
             tc.tile_pool(name="cp", bufs=2) as cp, \
             tc.tile_pool(name="gp", bufs=6) as gp, \
             tc.tile_pool(name="tp", bufs=2) as tp:
            ns_t = None
            for s in range(S):
                last = s == S - 1
                h_rd = h_t[(s + 1) % 2]   # h[s-1]
                h_wr = h_t[s % 2]         # h[s]
                if not last:
                    st = sp.tile([128, KD, BC], BF16, name=f"st{s}", tag="st")
                    nc.sync.dma_start(st[:], r3(slotsT[s]))
                    ct = cp.tile([128, KD, BC], BF16, name=f"ct{s}", tag="ct")
                    nc.sync.dma_start(ct[:], r3(cumT[s]))
                    dps = pp.tile([128, BC], F32, name=f"dps{s}", tag="ps")
                    mm_group(dps[:], [(ones_bf[0:1, 0:128],
                                       d1_sb[0:1, s * BC:(s + 1) * BC])])
                    d_bc = tp.tile([128, BC], F32, name=f"dbc{s}", tag="dbc",
                                   bufs=1)
                    nc.scalar.activation(d_bc[:], dps[:], AF.Copy)
                if s == 10:
                    nc.sync.dma_start(wo_sb[:], r3(woT))
                tsim = None
                if not last:
                    tsim = tp.tile([128, KD, BC], BF16, name=f"tm{s}", tag="tsim",
                                   bufs=1)
                    nc.vector.tensor_mul(tsim[:], st[:], ut[:])
                gates = [None] * KM
                for pos, m in enumerate(M_ORDER):
                    if pos == 4:
                        if not last:
                            sims_row(s + 1, tsim, str(s), tp)
                        if s == S - 2:
                            nc.scalar.activation(g_row[:], max_row[:], MSIG)
                            gps = pp.tile([128, BC], F32, name="gps", tag="ps")
                            mm_group(gps[:], [(ones_bf[0:1, 0:128], g_row[:])])
                            G_bc = tp.tile([128, BC], BF16, name="G_bc",
                                           tag="gbc", bufs=1)
                            nc.scalar.activation(G_bc[:], gps[:], AF.Copy)
                            ns_t = sp.tile([128, KD, BC], BF16, name="ns_t",
                                           tag="st")
                            for k in range(KD):
                                nc.vector.tensor_mul(ns_t[:, k, :], G_bc[:],
                                                     P_t[:, k, :])
                    sl = slice(128 * m, 128 * (m + 1))
                    ps = pp.tile([128, BC], F32, name=f"ps_{s}_{m}", tag="ps")
                    if last:
                        terms = [(aT_sb[:, k, sl], ns_t[:, k, :]) for k in range(KD)]
                    else:
                        terms = [(aT_sb[:, k, sl], st[:, k, :]) for k in range(KD)]
                        terms += [(cT_sb[:, k, sl], ct[:, k, :]) for k in range(KD)]

                    if s > 0:
                        terms += [(whT_sb[:, k, sl], h_rd[k][:]) for k in range(KD)]
                    mm_group(ps[:], terms)
                    if not last:
                        nc.vector.scalar_tensor_tensor(
                            ps[:], d_bc[:], d_sb[:, m:m + 1], ps[:],
                            mybir.AluOpType.mult, mybir.AluOpType.add)
                    nc.vector.tensor_add(ps[:], ps[:], shared_sb[:, m, :])
                    gt = gp.tile([128, BC], F32, name=f"g_{s}_{m}", tag="gate")
                    nc.scalar.activation(gt[:], ps[:], MTANH if m // 4 == 2 else MSIG)
                    gates[m] = gt
                    # after chunk j completes (i,f,g,o present), update c/h
                    j = m - 12
                    if j >= 0:
                        ig, fg, gg, og = (gates[j], gates[4 + j], gates[8 + j],
                                          gates[12 + j])
                        tct = tp.tile([128, BC], F32, name=f"t_{s}_{j}", tag="tct")
                        if s == 0:
                            nc.vector.tensor_mul(c_t[j][:], ig[:], gg[:])
                        else:
                            t2 = tp.tile([128, BC], F32, name=f"u_{s}_{j}", tag="t2")
                            nc.vector.tensor_mul(t2[:], fg[:], c_t[j][:])
                            nc.vector.tensor_mul(c_t[j][:], ig[:], gg[:])
                            nc.vector.tensor_add(c_t[j][:], c_t[j][:], t2[:])
                        nc.scalar.activation(tct[:], c_t[j][:], MTANH)
                        nc.vector.tensor_mul(h_wr[j][:], og[:], tct[:])
                if s in dbg:
                    cD, hD = dbg[s]
                    for j in range(KD):
                        nc.sync.dma_start(cD[128 * j:128 * (j + 1), :], c_t[j][:])
                        hcp = tp.tile([128, BC], F32, name=f"hcp{s}_{j}",
                                      tag="hcp")
                        nc.scalar.activation(hcp[:], h_wr[j][:], AF.Copy)
                        nc.sync.dma_start(hD[128 * j:128 * (j + 1), :], hcp[:])
                if not last:
                    # sims row for original slot s+1, while the tile is resident
                    tsim = tp.tile([128, KD, BC], BF16, name=f"tm{s}", tag="tsim",
                                   bufs=1)
                    nc.vector.tensor_mul(tsim[:], st[:], ut[:])
                    sims_row(s + 1, tsim, str(s), tp)
                if s == S - 2:
                    # g = sigmoid(max_s sims);  new_slot = g * P
                    nc.scalar.activation(g_row[:], max_row[:], MSIG)
                    gps = pp.tile([128, BC], F32, name="gps", tag="ps")
                    mm_group(gps[:], [(ones_bf[0:1, 0:128], g_row[:])])
                    G_bc = tp.tile([128, BC], BF16, name="G_bc", tag="gbc", bufs=1)
                    nc.scalar.activation(G_bc[:], gps[:], AF.Copy)
                    ns_t = sp.tile([128, KD, BC], BF16, name="ns_t", tag="st")
                    for k in range(KD):
                        nc.vector.tensor_mul(ns_t[:, k, :], G_bc[:], P_t[:, k, :])

        # ================= epilogue =================
        with tc.tile_pool(name="ep", bufs=1) as ep:
            hp_sb = ep.tile([128, KD, BC], F16, name="hp_sb")
            nc.sync.dma_start(hp_sb[:], r3(hpT))
            x16 = ep.tile([128, KD, BC], F16, name="x16")
            nc.scalar.activation(x16[:], xt[:].bitcast(F32), AF.Copy)
            for m in range(KD):
                sl = slice(128 * m, 128 * (m + 1))
                eps = pp.tile([128, BC], F32, name=f"eps{m}", tag="ps")
                terms = [(wo_sb[:, j, sl], h_t[(S - 1) % 2][j][:])
                         for j in range(KD)]
                terms += [(wo_sb[:, 4 + j, sl], hp_sb[:, j, :]) for j in range(KD)]
                terms += [(wo_sb[:, 8 + j, sl], x16[:, j, :]) for j in range(KD)]
                mm_group(eps[:], terms)
                out_t = ep.tile([128, BC], F32, name=f"o{m}", tag="out", bufs=2)
                nc.scalar.activation(out_t[:], eps[:], MTANH, bias=bo_sb[:, m:m + 1])
                nc.sync.dma_start(hnT[128 * m:128 * (m + 1), :], out_t[:])

        pp.release()
        st_p.release()
        wp.release()

    nc.compile()
    return nc


def kernel(**inputs):
    global _BUILT
    if _BUILT is None:
        _BUILT = _build_program()
    nc = _BUILT

    f32 = np.float32
    x = np.asarray(inputs["x_t"], f32)
    hp = np.asarray(inputs["h_prev"], f32)
    slots = np.asarray(inputs["slots"], f32)
    cum = np.asarray(inputs["cum_feats"], f32)
    dt = np.asarray(inputs["delta_t"], f32)
    Wk = np.asarray(inputs["Wk"], f32)
    Wq = np.asarray(inputs["Wq"], f32)
    Wv = np.asarray(inputs["Wv"], f32)
    bv = np.asarray(inputs["bv"], f32)
    Wr = np.asarray(inputs["Wr"], f32)
    br = np.asarray(inputs["br"], f32)
    Wa = np.asarray(inputs["Wa"], f32)
    ba = np.asarray(inputs["ba"], f32)
    W_ih = np.asarray(inputs["W_ih"], f32)
    W_hh = np.asarray(inputs["W_hh"], f32)
    b_ih = np.asarray(inputs["b_ih"], f32)
    b_hh = np.asarray(inputs["b_hh"], f32)
    Wo = np.asarray(inputs["Wo"], f32)
    bo = np.asarray(inputs["bo"], f32)

    xT = np.ascontiguousarray(x.T)
    hpT = hp.T.astype(np.float16)
    s0T = slots[:, 0, :].T.astype(BF)
    slotsT = slots[:, 1:, :].transpose(1, 2, 0).astype(BF)
    cumT = cum[:, 1:, :].transpose(1, 2, 0).astype(BF)
    d1T = np.ascontiguousarray((dt[:, 1:] + 1.0).T).astype(BF)

    m2T = np.ascontiguousarray(Wq.T @ Wk)
    wvT = np.ascontiguousarray(Wv.T)
    wrT = np.ascontiguousarray(Wr.T)
    waT = np.ascontiguousarray(Wa.T)
    aT = np.ascontiguousarray(W_ih[:, :D].T).astype(BF)
    cT = np.ascontiguousarray(W_ih[:, D:2 * D].T).astype(BF)
    d_row = W_ih[:, 2 * D].reshape(1, G4).astype(BF)
    bias_row = (b_ih + b_hh).reshape(1, G4).astype(BF)
    whT = W_hh.T.astype(np.float16)
    woT = Wo.T.astype(np.float16)

    shared_w = {
        "m2T": m2T, "wvT": wvT, "wrT": wrT, "waT": waT,
        "bv": bv.reshape(D, 1), "br": br.reshape(1, 1), "ba": ba.reshape(1, 1),
        "bo": bo.reshape(H, 1), "aT": aT, "cT": cT, "d_row": d_row,
        "bias_row": bias_row, "whT": whT, "woT": woT,
        "d_col": W_ih[:, 2 * D].reshape(G4, 1).copy(),
    }
    in_maps = []
    for c in range(NCORES):
        lo, hi = c * BC, (c + 1) * BC
        m = dict(shared_w)
        m["xT"] = xT[:, lo:hi]
        m["hpT"] = hpT[:, lo:hi]
        m["s0T"] = s0T[:, lo:hi]
        m["slotsT"] = slotsT[:, :, lo:hi]
        m["cumT"] = cumT[:, :, lo:hi]
        m["d1T"] = d1T[:, lo:hi].reshape(1, (S - 1) * BC)
        in_maps.append(m)

    res = bass_utils.run_bass_kernel_spmd(nc, in_maps, core_ids=list(range(NCORES)),
                                          **_RUN_KWARGS)
    global _LAST_RESULTS
    _LAST_RESULTS = res

    out = np.empty((B, H), np.float32)
    for c in range(NCORES):
        out[c * BC:(c + 1) * BC, :] = res.results[c]["hnT"].T
    return out


_RUN_KWARGS = {}
_LAST_RESULTS = None



# revision 2
# speedup vs baseline: 1.2982x; 1.2982x over previous
"""Trainium2 Bass kernel for nn_EventMemoryCell (B=4096, D=H=512, S=16).

Strategy (hardcoded for the spec shapes):
  - Data parallel over batch across 8 cores (512 rows each), parameters
    replicated; one SPMD NEFF.
  - Everything on-device runs in a transposed (feature-on-partition,
    batch-on-free) layout, so every matmul contracts over partitions and
    the LSTM recurrence needs no transposes.
  - mem_seq is never materialized: for s<15,
      xg[s] = A@slots_old[s+1] + C@cum_old[s+1] + d*(delta_old[s+1]+1)
              + shared,  shared = A@(leak*x) + 2*C@x + (b_ih+b_hh)
    and for s=15 xg[15] = A@new_slot + shared (cum part == 2x is in shared,
    delta part == 0).
  - Attention: sims = slots . ((Wk^T Wq) x), so keys (B,S,H) is never built.
  - The big per-step GEMMs (A/C over the slot/cum streams and W_hh over h)
    run in fp8-e4m3 with DoubleRow perf mode (2 fp8 k-rows per PE cell,
    2x throughput). Weights are pre-scaled x64 on the host so their 0.02-
    scale values clear the e4m3 denormal range; the gate activation applies
    scale=1/64 to undo it. The per-step "+ shared" lands in PSUM via an
    identity matmul and "+ d*delta" via a K=1 matmul, keeping the vector
    engine off the critical path.
  - Gate-tile emission is software-pipelined (stream-side matmuls of the
    next tiles are issued before the W_hh terms of earlier tiles) so the
    tensor engine doesn't stall on the h-recurrence tail.
"""
import sys

sys.path.insert(0, "/opt/trn_rl_repo")

import numpy as np
import ml_dtypes

import concourse.bass as bass
import concourse.tile as tile
import concourse.mybir as mybir
from concourse import bacc, bass_utils

F32 = mybir.dt.float32
F32R = mybir.dt.float32r
F16 = mybir.dt.float16
FP8 = mybir.dt.float8e4
AF = mybir.ActivationFunctionType
DR = mybir.MatmulPerfMode.DoubleRow
NPF16 = np.float16
NPF8 = ml_dtypes.float8_e4m3fn

B, S, D, H = 4096, 16, 512, 512
NCORES = 8
BC = B // NCORES            # 512 batch rows per core
G4 = 4 * H                  # 2048 gate rows
KD = D // 128               # 4 k-tiles over D/H
KM = G4 // 128              # 16 gate partition tiles
KO = (2 * H + D) // 128     # 12 k-tiles for the output projection
WSCALE = 64.0               # fp8 weight prescale (undone in gate activation)

# interleaved gate-tile order [0,4,8,12, 1,5,9,13, ...]: finish chunk j's
# i/f/g/o gates together so c/h updates start early
M_ORDER = [j + 4 * i for j in range(4) for i in range(4)]
PIPE = 3                    # gate-tile groups opened ahead of their Wh terms

_BUILT = None
DEBUG_STEPS = ()  # set before first kernel() call to dump c/h after these steps


def _build_program():
    nc = bacc.Bacc("TRN2", target_bir_lowering=False, debug=False)

    def din(name, shape, dt):
        return nc.dram_tensor(name, list(shape), dt, kind="ExternalInput").ap()

    xT = din("xT", (D, BC), F32R)
    s0T = din("s0T", (D, BC), F16)
    slotsT = din("slotsT", (S - 1, D, BC), FP8)
    cumT = din("cumT", (S - 1, D, BC), FP8)
    d1T = din("d1T", (1, (S - 1) * BC), F16)
    hpT = din("hpT", (H, BC), F16)
    m2T = din("m2T", (D, D), F32R)
    wvT = din("wvT", (D, D), F32R)
    wrT = din("wrT", (D, 1), F32R)
    waT = din("waT", (D, 1), F32R)
    bv = din("bv", (D, 1), F32)
    br = din("br", (1, 1), F32)
    ba = din("ba", (1, 1), F32)
    bo = din("bo", (H, 1), F32)
    aT = din("aT", (D, G4), F16)          # x64
    cT = din("cT", (D, G4), F16)          # x64
    bias_row = din("bias_row", (1, G4), F16)  # x64
    dr64 = din("dr64", (1, G4), F16)      # x64 delta column of W_ih
    a8 = din("a8", (D, G4), FP8)          # x64
    c8 = din("c8", (D, G4), FP8)          # x64
    wh8 = din("wh8", (H, G4), FP8)        # x64
    woT = din("woT", (2 * H + D, H), F16)
    eyeT = din("eyeT", (128, 128), F16)
    hnT = nc.dram_tensor("hnT", [H, BC], F32, kind="ExternalOutput").ap()
    dbg = {}
    for ds in DEBUG_STEPS:
        dbg[ds] = (nc.dram_tensor(f"cD{ds}", [H, BC], F32, kind="ExternalOutput").ap(),
                   nc.dram_tensor(f"hD{ds}", [H, BC], F32, kind="ExternalOutput").ap())

    r3 = lambda ap: ap.rearrange("(kt p) b -> p kt b", p=128)
    r2 = lambda ap: ap.rearrange("(kt p) o -> p (kt o)", p=128)

    with tile.TileContext(nc) as tc:
        wp = tc.alloc_tile_pool(name="wp", bufs=1)
        st_p = tc.alloc_tile_pool(name="state", bufs=1)
        pp = tc.alloc_tile_pool(name="pp", bufs=8, space="PSUM")

        # ---- resident weights / constants ----
        a8_sb = wp.tile([128, KD, G4], FP8, name="a8_sb")
        c8_sb = wp.tile([128, KD, G4], FP8, name="c8_sb")
        wh8_sb = wp.tile([128, KD, G4], FP8, name="wh8_sb")
        eye_sb = wp.tile([128, 128], F16, name="eye_sb")
        nc.sync.dma_start(eye_sb[:], eyeT)
        dr_sb = wp.tile([1, G4], F16, name="dr_sb")
        nc.sync.dma_start(dr_sb[:], dr64)
        d1_sb = wp.tile([1, (S - 1) * BC], F16, name="d1_sb")
        nc.sync.dma_start(d1_sb[:], d1T)
        wo_sb = wp.tile([128, KO, H], F16, name="wo_sb")
        bo_sb = wp.tile([128, KD], F32, name="bo_sb")
        nc.sync.dma_start(bo_sb[:], r2(bo))
        ones_bf = wp.tile([1, BC], F16, name="ones_bf")
        nc.vector.memset(ones_bf[:], 1.0)
        ones_col = wp.tile([128, 1], F16, name="ones_col")
        nc.vector.memset(ones_col[:], 1.0)

        xt = st_p.tile([128, KD, BC], F32R, name="xt")
        nc.sync.dma_start(xt[:], r3(xT))
        shared_sb = st_p.tile([128, KM, BC], F16, name="shared_sb")
        ut = st_p.tile([128, KD, BC], F16, name="ut")
        P_t = st_p.tile([128, KD, BC], F16, name="P_t")
        c_t = [st_p.tile([128, BC], F32, name=f"c{k}", tag=f"c{k}") for k in range(KD)]
        h8 = [st_p.tile([128, KD, BC], FP8, name=f"h8_{pq}", tag=f"h8_{pq}")
              for pq in range(2)]
        h15 = st_p.tile([128, KD, BC], F16, name="h15")
        g_row = st_p.tile([1, BC], F16, name="g_row")
        max_row = st_p.tile([1, BC], F32, name="max_row")

        MSIG, MTANH = AF.Sigmoid, AF.Tanh

        def sims_row(idx, tsrc):
            # running max over slots: max_row = max(max_row, slots_s . u)
            srp = pp.tile([128, BC], F32, name=f"srp{idx}", tag="ps")
            mm_group(srp[0:1, :], [(ones_col[:], tsrc[:, k, :]) for k in range(KD)])
            if idx == 0:
                nc.scalar.activation(max_row[:], srp[0:1, :], AF.Copy)
            else:
                nc.vector.tensor_max(max_row[:], max_row[:], srp[0:1, :])

        def mm_group(ps_ap, terms):
            n = len(terms)
            for i, term in enumerate(terms):
                pm = term[2] if len(term) > 2 else None
                nc.tensor.matmul(ps_ap, term[0], term[1], start=(i == 0),
                                 stop=(i == n - 1), perf_mode=pm)

        # ================= prologue =================
        with tc.tile_pool(name="prop", bufs=1) as prop:
            m2_sb = prop.tile([128, KD, D], F32R, name="m2_sb")
            nc.sync.dma_start(m2_sb[:], r3(m2T))
            wv_sb = prop.tile([128, KD, D], F32R, name="wv_sb")
            nc.sync.dma_start(wv_sb[:], r3(wvT))
            wr_sb = prop.tile([128, KD], F32R, name="wr_sb")
            nc.sync.dma_start(wr_sb[:], r2(wrT))
            wa_sb = prop.tile([128, KD], F32R, name="wa_sb")
            nc.sync.dma_start(wa_sb[:], r2(waT))
            bv_sb = prop.tile([128, KD], F32, name="bv_sb")
            nc.sync.dma_start(bv_sb[:], r2(bv))
            br_sb = prop.tile([1, 1], F32, name="br_sb")
            nc.sync.dma_start(br_sb[:], br)
            ba_sb = prop.tile([1, 1], F32, name="ba_sb")
            nc.sync.dma_start(ba_sb[:], ba)
            s0_sb = prop.tile([128, KD, BC], F16, name="s0_sb")
            nc.sync.dma_start(s0_sb[:], r3(s0T))
            aT_sb = prop.tile([128, KD, G4], F16, name="aT_sb")
            nc.sync.dma_start(aT_sb[:], r3(aT))
            cT_sb = prop.tile([128, KD, G4], F16, name="cT_sb")
            nc.sync.dma_start(cT_sb[:], r3(cT))
            bias_sb = prop.tile([1, G4], F16, name="bias_sb")
            nc.sync.dma_start(bias_sb[:], bias_row)
            # big resident weights: emitted after the prologue-critical DMAs
            nc.sync.dma_start(a8_sb[:], r3(a8))
            nc.sync.dma_start(c8_sb[:], r3(c8))
            nc.sync.dma_start(wh8_sb[:], r3(wh8))
            vt = prop.tile([128, KD, BC], F16, name="vt")
            lx = prop.tile([128, KD, BC], F16, name="lx")
            xbf = prop.tile([128, KD, BC], F16, name="xbf")
            x2bf = prop.tile([128, KD, BC], F16, name="x2bf")
            r_row = prop.tile([1, BC], F16, name="r_row")
            lk_row = prop.tile([1, BC], F16, name="lk_row")
            R_bc = prop.tile([128, BC], F16, name="R_bc")
            L_bc = prop.tile([128, BC], F16, name="L_bc")

            nc.scalar.activation(xbf[:], xt[:].bitcast(F32), AF.Copy)
            nc.scalar.activation(x2bf[:], xt[:].bitcast(F32), AF.Copy, scale=2.0)

            # u = (Wk^T Wq) x ; v = Wv x + bv   (feature-major)
            for m in range(KD):
                ups = pp.tile([128, BC], F32, name=f"ups{m}", tag="ps")
                mm_group(ups[:], [(m2_sb[:, k, 128 * m:128 * (m + 1)], xt[:, k, :])
                                  for k in range(KD)])
                nc.scalar.activation(ut[:, m, :], ups[:], AF.Copy)
            for m in range(KD):
                vps = pp.tile([128, BC], F32, name=f"vps{m}", tag="ps")
                mm_group(vps[:], [(wv_sb[:, k, 128 * m:128 * (m + 1)], xt[:, k, :])
                                  for k in range(KD)])
                nc.scalar.activation(vt[:, m, :], vps[:], AF.Identity,
                                     bias=bv_sb[:, m:m + 1])

            # r / leak rows
            rps = pp.tile([128, BC], F32, name="rps", tag="ps")
            mm_group(rps[0:1, :], [(wr_sb[:, k:k + 1], xt[:, k, :]) for k in range(KD)])
            nc.scalar.activation(r_row[:], rps[0:1, :], MSIG, bias=br_sb[0:1, 0:1])
            lps = pp.tile([128, BC], F32, name="lps", tag="ps")
            mm_group(lps[0:1, :], [(wa_sb[:, k:k + 1], xt[:, k, :]) for k in range(KD)])
            nc.scalar.activation(lk_row[:], lps[0:1, :], MSIG, bias=ba_sb[0:1, 0:1])

            # broadcast r/leak rows to 128 partitions via a K=1 matmul
            bps = pp.tile([128, BC], F32, name="bps", tag="ps")
            mm_group(bps[:], [(ones_bf[0:1, 0:128], r_row[:])])
            nc.scalar.activation(R_bc[:], bps[:], AF.Copy)
            bps2 = pp.tile([128, BC], F32, name="bps2", tag="ps")
            mm_group(bps2[:], [(ones_bf[0:1, 0:128], lk_row[:])])
            nc.scalar.activation(L_bc[:], bps2[:], AF.Copy)

            # P = r*slots0 + (1-r)*v = v + r*(slots0 - v);  lx = leak*x
            for k in range(KD):
                t1 = prop.tile([128, BC], F16, name=f"pt{k}", tag="ptmp", bufs=2)
                nc.vector.tensor_sub(t1[:], s0_sb[:, k, :], vt[:, k, :])
                nc.vector.tensor_mul(t1[:], R_bc[:], t1[:])
                nc.vector.tensor_add(P_t[:, k, :], vt[:, k, :], t1[:])
                nc.vector.tensor_mul(lx[:, k, :], L_bc[:], xbf[:, k, :])

            # sims row 0 (original slot 0)
            ts0 = prop.tile([128, KD, BC], F16, name="ts0")
            nc.vector.tensor_mul(ts0[:], s0_sb[:], ut[:])
            sims_row(0, ts0)

            # shared = A@lx + C@(2x) + (b_ih+b_hh) x ones   (all x64)
            for m in range(KM):
                sl = slice(128 * m, 128 * (m + 1))
                sps = pp.tile([128, BC], F32, name=f"sps{m}", tag="ps")
                terms = [(aT_sb[:, k, sl], lx[:, k, :]) for k in range(KD)]
                terms += [(cT_sb[:, k, sl], x2bf[:, k, :]) for k in range(KD)]
                terms += [(bias_sb[0:1, sl], ones_bf[:])]
                mm_group(sps[:], terms)
                nc.scalar.activation(shared_sb[:, m, :], sps[:], AF.Copy)

        # ================= LSTM over S steps =================
        with tc.tile_pool(name="sp", bufs=2) as sp, \
             tc.tile_pool(name="cp", bufs=2) as cp, \
             tc.tile_pool(name="gp", bufs=10) as gp, \
             tc.tile_pool(name="tp", bufs=2) as tp:
            ns_t = None
            for s in range(S):
                last = s == S - 1
                h_rd = h8[(s + 1) % 2]   # h[s-1]
                h_wr = h8[s % 2] if not last else h15
                if not last:
                    st = sp.tile([128, KD, BC], FP8, name=f"st{s}", tag="st")
                    nc.sync.dma_start(st[:], r3(slotsT[s]))
                    ct = cp.tile([128, KD, BC], FP8, name=f"ct{s}", tag="ct")
                    nc.sync.dma_start(ct[:], r3(cumT[s]))
                if s == 10:
                    nc.sync.dma_start(wo_sb[:], r3(woT))

                gates = [None] * KM

                def emit_pre(m):
                    sl = slice(128 * m, 128 * (m + 1))
                    ps = pp.tile([128, BC], F32, name=f"ps_{s}_{m}", tag="ps")
                    pre = [(eye_sb[:], shared_sb[:, m, :])]
                    if not last:
                        pre.append((dr_sb[0:1, sl],
                                    d1_sb[0:1, s * BC:(s + 1) * BC]))
                    src = ns_t if last else st
                    for t in (0, 1):
                        pre.append((a8_sb[:, 2 * t:2 * t + 2, sl],
                                    src[:, 2 * t:2 * t + 2, :], DR))
                    if not last:
                        for t in (0, 1):
                            pre.append((c8_sb[:, 2 * t:2 * t + 2, sl],
                                        ct[:, 2 * t:2 * t + 2, :], DR))
                    fin = []
                    if s > 0:
                        for t in (0, 1):
                            fin.append((wh8_sb[:, 2 * t:2 * t + 2, sl],
                                        h_rd[:, 2 * t:2 * t + 2, :], DR))
                    n_all = len(pre) + len(fin)
                    for i, term in enumerate(pre):
                        pm = term[2] if len(term) > 2 else None
                        nc.tensor.matmul(ps[:], term[0], term[1],
                                         start=(i == 0), stop=(i == n_all - 1),
                                         perf_mode=pm)
                    return ps, fin, len(pre)

                def emit_fin(m, ps, fin, npre):
                    n_all = npre + len(fin)
                    for i, term in enumerate(fin):
                        pm = term[2] if len(term) > 2 else None
                        nc.tensor.matmul(ps[:], term[0], term[1], start=False,
                                         stop=(npre + i == n_all - 1),
                                         perf_mode=pm)
                    gt = gp.tile([128, BC], F16, name=f"g_{s}_{m}", tag="gate")
                    nc.scalar.activation(gt[:], ps[:],
                                         MTANH if m // 4 == 2 else MSIG,
                                         scale=1.0 / WSCALE)
                    gates[m] = gt
                    # after chunk j completes (i,f,g,o present), update c/h
                    j = m - 12
                    if j >= 0:
                        ig, fg, gg, og = (gates[j], gates[4 + j], gates[8 + j],
                                          gates[12 + j])
                        tct = tp.tile([128, BC], F16, name=f"t_{s}_{j}", tag="tct")
                        if s == 0:
                            nc.vector.tensor_mul(c_t[j][:], ig[:], gg[:])
                        else:
                            t2 = tp.tile([128, BC], F32, name=f"u_{s}_{j}", tag="t2")
                            nc.vector.tensor_mul(t2[:], fg[:], c_t[j][:])
                            nc.vector.tensor_mul(c_t[j][:], ig[:], gg[:])
                            nc.vector.tensor_add(c_t[j][:], c_t[j][:], t2[:])
                        nc.scalar.activation(tct[:], c_t[j][:], MTANH)
                        nc.vector.tensor_mul(h_wr[:, j, :], og[:], tct[:])

                pend = []
                for m in M_ORDER:
                    pend.append((m, *emit_pre(m)))
                    if len(pend) > PIPE:
                        emit_fin(*pend.pop(0))
                while pend:
                    emit_fin(*pend.pop(0))

                if s in dbg:
                    cD, hD = dbg[s]
                    for j in range(KD):
                        nc.sync.dma_start(cD[128 * j:128 * (j + 1), :], c_t[j][:])
                        hcp = tp.tile([128, BC], F32, name=f"hcp{s}_{j}",
                                      tag="hcp")
                        nc.scalar.activation(hcp[:], h_wr[:, j, :], AF.Copy)
                        nc.sync.dma_start(hD[128 * j:128 * (j + 1), :], hcp[:])
                if not last:
                    # sims row for original slot s+1 (emitted at end of step:
                    # the ones-matmuls fill the PE while the h-tail resolves)
                    tsim = tp.tile([128, KD, BC], F16, name=f"tm{s}", tag="tsim",
                                   bufs=1)
                    nc.vector.tensor_mul(tsim[:], st[:], ut[:])
                    sims_row(s + 1, tsim)
                if s == S - 2:
                    # g = sigmoid(max_s sims);  new_slot = g * P
                    nc.scalar.activation(g_row[:], max_row[:], MSIG)
                    gps = pp.tile([128, BC], F32, name="gps", tag="ps")
                    mm_group(gps[:], [(ones_bf[0:1, 0:128], g_row[:])])
                    G_bc = tp.tile([128, BC], F16, name="G_bc", tag="gbc", bufs=1)
                    nc.scalar.activation(G_bc[:], gps[:], AF.Copy)
                    ns_t = sp.tile([128, KD, BC], FP8, name="ns_t", tag="st")
                    for k in range(KD):
                        nc.vector.tensor_mul(ns_t[:, k, :], G_bc[:], P_t[:, k, :])

        # ================= epilogue =================
        with tc.tile_pool(name="ep", bufs=1) as ep:
            hp_sb = ep.tile([128, KD, BC], F16, name="hp_sb")
            nc.sync.dma_start(hp_sb[:], r3(hpT))
            x16 = ep.tile([128, KD, BC], F16, name="x16")
            nc.scalar.activation(x16[:], xt[:].bitcast(F32), AF.Copy)
            for m in range(KD):
                sl = slice(128 * m, 128 * (m + 1))
                eps = pp.tile([128, BC], F32, name=f"eps{m}", tag="ps")
                terms = [(wo_sb[:, j, sl], h15[:, j, :]) for j in range(KD)]
                terms += [(wo_sb[:, 4 + j, sl], hp_sb[:, j, :]) for j in range(KD)]
                terms += [(wo_sb[:, 8 + j, sl], x16[:, j, :]) for j in range(KD)]
                mm_group(eps[:], terms)
                out_t = ep.tile([128, BC], F32, name=f"o{m}", tag="out", bufs=2)
                nc.scalar.activation(out_t[:], eps[:], MTANH, bias=bo_sb[:, m:m + 1])
                nc.sync.dma_start(hnT[128 * m:128 * (m + 1), :], out_t[:])

        pp.release()
        st_p.release()
        wp.release()

    nc.compile()
    return nc


def kernel(**inputs):
    global _BUILT
    if _BUILT is None:
        _BUILT = _build_program()
    nc = _BUILT

    f32 = np.float32
    x = np.asarray(inputs["x_t"], f32)
    hp = np.asarray(inputs["h_prev"], f32)
    slots = np.asarray(inputs["slots"], f32)
    cum = np.asarray(inputs["cum_feats"], f32)
    dt = np.asarray(inputs["delta_t"], f32)
    Wk = np.asarray(inputs["Wk"], f32)
    Wq = np.asarray(inputs["Wq"], f32)
    Wv = np.asarray(inputs["Wv"], f32)
    bv = np.asarray(inputs["bv"], f32)
    Wr = np.asarray(inputs["Wr"], f32)
    br = np.asarray(inputs["br"], f32)
    Wa = np.asarray(inputs["Wa"], f32)
    ba = np.asarray(inputs["ba"], f32)
    W_ih = np.asarray(inputs["W_ih"], f32)
    W_hh = np.asarray(inputs["W_hh"], f32)
    b_ih = np.asarray(inputs["b_ih"], f32)
    b_hh = np.asarray(inputs["b_hh"], f32)
    Wo = np.asarray(inputs["Wo"], f32)
    bo = np.asarray(inputs["bo"], f32)

    xT = np.ascontiguousarray(x.T)
    hpT = hp.T.astype(NPF16)
    s0T = slots[:, 0, :].T.astype(NPF16)
    slotsT = slots[:, 1:, :].transpose(1, 2, 0).astype(NPF8)
    cumT = cum[:, 1:, :].transpose(1, 2, 0).astype(NPF8)
    d1T = np.ascontiguousarray((dt[:, 1:] + 1.0).T).astype(NPF16)

    m2T = np.ascontiguousarray(Wq.T @ Wk)
    wvT = np.ascontiguousarray(Wv.T)
    wrT = np.ascontiguousarray(Wr.T)
    waT = np.ascontiguousarray(Wa.T)
    A64 = WSCALE * W_ih[:, :D]
    C64 = WSCALE * W_ih[:, D:2 * D]
    aT = np.ascontiguousarray(A64.T).astype(NPF16)
    cT = np.ascontiguousarray(C64.T).astype(NPF16)
    a8 = np.ascontiguousarray(A64.T).astype(NPF8)
    c8 = np.ascontiguousarray(C64.T).astype(NPF8)
    wh8 = np.ascontiguousarray((WSCALE * W_hh).T).astype(NPF8)
    dr64 = (WSCALE * W_ih[:, 2 * D]).reshape(1, G4).astype(NPF16)
    bias_row = (WSCALE * (b_ih + b_hh)).reshape(1, G4).astype(NPF16)
    woT = Wo.T.astype(NPF16)
    eye = np.eye(128, dtype=NPF16)

    shared_w = {
        "m2T": m2T, "wvT": wvT, "wrT": wrT, "waT": waT,
        "bv": bv.reshape(D, 1), "br": br.reshape(1, 1), "ba": ba.reshape(1, 1),
        "bo": bo.reshape(H, 1), "aT": aT, "cT": cT, "dr64": dr64,
        "bias_row": bias_row, "a8": a8, "c8": c8, "wh8": wh8, "woT": woT,
        "eyeT": eye,
    }
    in_maps = []
    for c in range(NCORES):
        lo, hi = c * BC, (c + 1) * BC
        m = dict(shared_w)
        m["xT"] = xT[:, lo:hi]
        m["hpT"] = hpT[:, lo:hi]
        m["s0T"] = s0T[:, lo:hi]
        m["slotsT"] = slotsT[:, :, lo:hi]
        m["cumT"] = cumT[:, :, lo:hi]
        m["d1T"] = d1T[:, lo:hi].reshape(1, (S - 1) * BC)
        in_maps.append(m)

    res = bass_utils.run_bass_kernel_spmd(nc, in_maps, core_ids=list(range(NCORES)),
                                          **_RUN_KWARGS)
    global _LAST_RESULTS
    _LAST_RESULTS = res

    out = np.empty((B, H), np.float32)
    for c in range(NCORES):
        out[c * BC:(c + 1) * BC, :] = res.results[c]["hnT"].T
    return out


_RUN_KWARGS = {}
_LAST_RESULTS = None


# revision 13
# speedup vs baseline: 1.4713x; 1.1333x over previous
"""Trainium2 Bass kernel for nn_EventMemoryCell (B=4096, D=H=512, S=16).

Strategy (hardcoded for the spec shapes):
  - Data parallel over batch across 8 cores (512 rows each), parameters
    replicated; one SPMD NEFF.
  - Everything on-device runs in a transposed (feature-on-partition,
    batch-on-free) layout, so every matmul contracts over partitions and
    the LSTM recurrence needs no transposes.
  - mem_seq is never materialized: for s<15,
      xg[s] = A@slots_old[s+1] + C@cum_old[s+1] + d*(delta_old[s+1]+1)
              + shared,  shared = A@(leak*x) + 2*C@x + (b_ih+b_hh)
    and for s=15 xg[15] = A@new_slot + shared (cum part == 2x is in shared,
    delta part == 0).
  - Attention: sims = slots . ((Wk^T Wq) x), so keys (B,S,H) is never built.
  - The big per-step GEMMs (A/C over the slot/cum streams and W_hh over h)
    run in fp8-e4m3 with DoubleRow perf mode (2 fp8 k-rows per PE cell,
    2x throughput). Weights are pre-scaled x64 on the host so their 0.02-
    scale values clear the e4m3 denormal range; the gate activation applies
    scale=1/64 to undo it. The per-step "+ shared" lands in PSUM via an
    identity matmul and "+ d*delta" via a K=1 matmul, keeping the vector
    engine off the critical path.
  - Gate-tile emission is software-pipelined (stream-side matmuls of the
    next tiles are issued before the W_hh terms of earlier tiles) so the
    tensor engine doesn't stall on the h-recurrence tail.
"""
import sys

sys.path.insert(0, "/opt/trn_rl_repo")

import numpy as np
import ml_dtypes

import concourse.bass as bass
import concourse.tile as tile
import concourse.mybir as mybir
from concourse import bacc, bass_utils

F32 = mybir.dt.float32
F32R = mybir.dt.float32r
F16 = mybir.dt.float16
FP8 = mybir.dt.float8e4
AF = mybir.ActivationFunctionType
DR = mybir.MatmulPerfMode.DoubleRow
NPF16 = np.float16
NPF8 = ml_dtypes.float8_e4m3fn

B, S, D, H = 4096, 16, 512, 512
NCORES = 8
BC = B // NCORES            # 512 batch rows per core
G4 = 4 * H                  # 2048 gate rows
KD = D // 128               # 4 k-tiles over D/H
KM = G4 // 128              # 16 gate partition tiles
KO = (2 * H + D) // 128     # 12 k-tiles for the output projection
WSCALE = 64.0               # fp8 weight prescale (undone in gate activation)

# interleaved gate-tile order [0,4,8,12, 1,5,9,13, ...]: finish chunk j's
# i/f/g/o gates together so c/h updates start early
M_ORDER = [j + 4 * i for j in range(4) for i in range(4)]
PIPE = 3                    # gate-tile groups opened ahead of their Wh terms

_BUILT = None
DEBUG_STEPS = ()  # set before first kernel() call to dump c/h after these steps


def _build_program():
    nc = bacc.Bacc("TRN2", target_bir_lowering=False, debug=False)

    def din(name, shape, dt):
        return nc.dram_tensor(name, list(shape), dt, kind="ExternalInput").ap()

    xT = din("xT", (D, BC), F32R)
    s0T = din("s0T", (D, BC), F16)
    slotsT = din("slotsT", (S - 1, D, BC), FP8)
    cumT = din("cumT", (S - 1, D, BC), FP8)
    d1T = din("d1T", (1, (S - 1) * BC), F16)
    hpT = din("hpT", (H, BC), F16)
    m2T = din("m2T", (D, D), F32R)
    wvT = din("wvT", (D, D), F32R)
    wrT = din("wrT", (D, 1), F32R)
    waT = din("waT", (D, 1), F32R)
    bv = din("bv", (D, 1), F32)
    br = din("br", (1, 1), F32)
    ba = din("ba", (1, 1), F32)
    bo = din("bo", (H, 1), F32)
    aT = din("aT", (D, G4), F16)          # x64
    cT = din("cT", (D, G4), F16)          # x64
    bias_row = din("bias_row", (1, G4), F16)  # x64
    d_col = din("d_col", (G4, 1), F32)    # x64 delta column of W_ih
    a8 = din("a8", (D, G4), FP8)          # x64
    c8 = din("c8", (D, G4), FP8)          # x64
    wh8 = din("wh8", (H, G4), FP8)        # x64
    woT = din("woT", (2 * H + D, H), F16)
    hnT = nc.dram_tensor("hnT", [H, BC], F32, kind="ExternalOutput").ap()
    dbg = {}
    for ds in DEBUG_STEPS:
        dbg[ds] = (nc.dram_tensor(f"cD{ds}", [H, BC], F32, kind="ExternalOutput").ap(),
                   nc.dram_tensor(f"hD{ds}", [H, BC], F32, kind="ExternalOutput").ap())

    r3 = lambda ap: ap.rearrange("(kt p) b -> p kt b", p=128)
    r2 = lambda ap: ap.rearrange("(kt p) o -> p (kt o)", p=128)

    with tile.TileContext(nc) as tc:
        wp = tc.alloc_tile_pool(name="wp", bufs=1)
        st_p = tc.alloc_tile_pool(name="state", bufs=1)
        pp = tc.alloc_tile_pool(name="pp", bufs=8, space="PSUM")

        # ---- resident weights / constants ----
        a8_sb = wp.tile([128, KD, G4], FP8, name="a8_sb")
        c8_sb = wp.tile([128, KD, G4], FP8, name="c8_sb")
        wh8_sb = wp.tile([128, KD, G4], FP8, name="wh8_sb")
        d_sb = wp.tile([128, KM], F32, name="d_sb")
        nc.sync.dma_start(d_sb[:], r2(d_col))
        d1_sb = wp.tile([1, (S - 1) * BC], F16, name="d1_sb")
        nc.sync.dma_start(d1_sb[:], d1T)
        wo_sb = wp.tile([128, KO, H], F16, name="wo_sb")
        bo_sb = wp.tile([128, KD], F32, name="bo_sb")
        nc.sync.dma_start(bo_sb[:], r2(bo))
        ones_bf = wp.tile([1, BC], F16, name="ones_bf")
        nc.vector.memset(ones_bf[:], 1.0)
        ones8 = wp.tile([128, 2, 16], FP8, name="ones8")
        nc.vector.memset(ones8[:], 1.0)

        xt = st_p.tile([128, KD, BC], F32R, name="xt")
        nc.sync.dma_start(xt[:], r3(xT))
        shared_sb = st_p.tile([128, KM, BC], F16, name="shared_sb")
        ut = st_p.tile([128, KD, BC], FP8, name="ut")
        P_t = st_p.tile([128, KD, BC], F16, name="P_t")
        c_t = [st_p.tile([128, BC], F32, name=f"c{k}", tag=f"c{k}") for k in range(KD)]
        h8 = [st_p.tile([128, KD, BC], FP8, name=f"h8_{pq}", tag=f"h8_{pq}")
              for pq in range(2)]
        h15 = st_p.tile([128, KD, BC], F16, name="h15")
        g_row = st_p.tile([1, BC], F16, name="g_row")
        max_row = st_p.tile([1, BC], F32, name="max_row")

        MSIG, MTANH = AF.Sigmoid, AF.Tanh

        def sims_row(idx, tsrc):
            # running max over slots: max_row = max(max_row, slots_s . u)
            srp = pp.tile([128, BC], F32, name=f"srp{idx}", tag="ps")
            mm_group(srp[0:1, :], [(ones8[:, :, 0:1], tsrc[:, 2 * t:2 * t + 2, :], DR)
                                   for t in (0, 1)])
            if idx == 0:
                nc.scalar.activation(max_row[:], srp[0:1, :], AF.Copy)
            else:
                nc.vector.tensor_max(max_row[:], max_row[:], srp[0:1, :])

        def mm_group(ps_ap, terms):
            n = len(terms)
            for i, term in enumerate(terms):
                pm = term[2] if len(term) > 2 else None
                nc.tensor.matmul(ps_ap, term[0], term[1], start=(i == 0),
                                 stop=(i == n - 1), perf_mode=pm)

        # ================= prologue =================
        with tc.tile_pool(name="prop", bufs=1) as prop:
            m2_sb = prop.tile([128, KD, D], F32R, name="m2_sb")
            nc.sync.dma_start(m2_sb[:], r3(m2T))
            wv_sb = prop.tile([128, KD, D], F32R, name="wv_sb")
            nc.sync.dma_start(wv_sb[:], r3(wvT))
            wr_sb = prop.tile([128, KD], F32R, name="wr_sb")
            nc.sync.dma_start(wr_sb[:], r2(wrT))
            wa_sb = prop.tile([128, KD], F32R, name="wa_sb")
            nc.sync.dma_start(wa_sb[:], r2(waT))
            bv_sb = prop.tile([128, KD], F32, name="bv_sb")
            nc.sync.dma_start(bv_sb[:], r2(bv))
            br_sb = prop.tile([1, 1], F32, name="br_sb")
            nc.sync.dma_start(br_sb[:], br)
            ba_sb = prop.tile([1, 1], F32, name="ba_sb")
            nc.sync.dma_start(ba_sb[:], ba)
            s0_sb = prop.tile([128, KD, BC], F16, name="s0_sb")
            nc.sync.dma_start(s0_sb[:], r3(s0T))
            aT_sb = prop.tile([128, KD, G4], F16, name="aT_sb")
            nc.sync.dma_start(aT_sb[:], r3(aT))
            cT_sb = prop.tile([128, KD, G4], F16, name="cT_sb")
            nc.sync.dma_start(cT_sb[:], r3(cT))
            bias_sb = prop.tile([1, G4], F16, name="bias_sb")
            nc.sync.dma_start(bias_sb[:], bias_row)
            # big resident weights: emitted after the prologue-critical DMAs
            nc.sync.dma_start(a8_sb[:], r3(a8))
            nc.sync.dma_start(c8_sb[:], r3(c8))
            nc.sync.dma_start(wh8_sb[:], r3(wh8))
            vt = prop.tile([128, KD, BC], F16, name="vt")
            lx = prop.tile([128, KD, BC], F16, name="lx")
            xbf = prop.tile([128, KD, BC], F16, name="xbf")
            x2bf = prop.tile([128, KD, BC], F16, name="x2bf")
            r_row = prop.tile([1, BC], F16, name="r_row")
            lk_row = prop.tile([1, BC], F16, name="lk_row")
            R_bc = prop.tile([128, BC], F16, name="R_bc")
            L_bc = prop.tile([128, BC], F16, name="L_bc")

            nc.scalar.activation(xbf[:], xt[:].bitcast(F32), AF.Copy)
            nc.scalar.activation(x2bf[:], xt[:].bitcast(F32), AF.Copy, scale=2.0)

            # u = (Wk^T Wq) x ; v = Wv x + bv   (feature-major)
            for m in range(KD):
                ups = pp.tile([128, BC], F32, name=f"ups{m}", tag="ps")
                mm_group(ups[:], [(m2_sb[:, k, 128 * m:128 * (m + 1)], xt[:, k, :])
                                  for k in range(KD)])
                nc.scalar.activation(ut[:, m, :], ups[:], AF.Copy)
            for m in range(KD):
                vps = pp.tile([128, BC], F32, name=f"vps{m}", tag="ps")
                mm_group(vps[:], [(wv_sb[:, k, 128 * m:128 * (m + 1)], xt[:, k, :])
                                  for k in range(KD)])
                nc.scalar.activation(vt[:, m, :], vps[:], AF.Identity,
                                     bias=bv_sb[:, m:m + 1])

            # r / leak rows
            rps = pp.tile([128, BC], F32, name="rps", tag="ps")
            mm_group(rps[0:1, :], [(wr_sb[:, k:k + 1], xt[:, k, :]) for k in range(KD)])
            nc.scalar.activation(r_row[:], rps[0:1, :], MSIG, bias=br_sb[0:1, 0:1])
            lps = pp.tile([128, BC], F32, name="lps", tag="ps")
            mm_group(lps[0:1, :], [(wa_sb[:, k:k + 1], xt[:, k, :]) for k in range(KD)])
            nc.scalar.activation(lk_row[:], lps[0:1, :], MSIG, bias=ba_sb[0:1, 0:1])

            # broadcast r/leak rows to 128 partitions via a K=1 matmul
            bps = pp.tile([128, BC], F32, name="bps", tag="ps")
            mm_group(bps[:], [(ones_bf[0:1, 0:128], r_row[:])])
            nc.scalar.activation(R_bc[:], bps[:], AF.Copy)
            bps2 = pp.tile([128, BC], F32, name="bps2", tag="ps")
            mm_group(bps2[:], [(ones_bf[0:1, 0:128], lk_row[:])])
            nc.scalar.activation(L_bc[:], bps2[:], AF.Copy)

            # P = r*slots0 + (1-r)*v = v + r*(slots0 - v);  lx = leak*x
            for k in range(KD):
                t1 = prop.tile([128, BC], F16, name=f"pt{k}", tag="ptmp", bufs=2)
                nc.vector.tensor_sub(t1[:], s0_sb[:, k, :], vt[:, k, :])
                nc.vector.tensor_mul(t1[:], R_bc[:], t1[:])
                nc.vector.tensor_add(P_t[:, k, :], vt[:, k, :], t1[:])
                nc.vector.tensor_mul(lx[:, k, :], L_bc[:], xbf[:, k, :])

            # sims row 0 (original slot 0)
            ts0 = prop.tile([128, KD, BC], FP8, name="ts0")
            nc.vector.tensor_mul(ts0[:], s0_sb[:], ut[:])
            sims_row(0, ts0)

            # shared = A@lx + C@(2x) + (b_ih+b_hh) x ones   (all x64)
            for m in range(KM):
                sl = slice(128 * m, 128 * (m + 1))
                sps = pp.tile([128, BC], F32, name=f"sps{m}", tag="ps")
                terms = [(aT_sb[:, k, sl], lx[:, k, :]) for k in range(KD)]
                terms += [(cT_sb[:, k, sl], x2bf[:, k, :]) for k in range(KD)]
                terms += [(bias_sb[0:1, sl], ones_bf[:])]
                mm_group(sps[:], terms)
                nc.scalar.activation(shared_sb[:, m, :], sps[:], AF.Copy)

        # ================= LSTM over S steps =================
        with tc.tile_pool(name="sp", bufs=2) as sp, \
             tc.tile_pool(name="cp", bufs=2) as cp, \
             tc.tile_pool(name="gp", bufs=10) as gp, \
             tc.tile_pool(name="tp", bufs=2) as tp:
            ns_t = None
            for s in range(S):
                last = s == S - 1
                h_rd = h8[(s + 1) % 2]   # h[s-1]
                h_wr = h8[s % 2] if not last else h15
                if not last:
                    st = sp.tile([128, KD, BC], FP8, name=f"st{s}", tag="st")
                    nc.sync.dma_start(st[:], r3(slotsT[s]))
                    ct = cp.tile([128, KD, BC], FP8, name=f"ct{s}", tag="ct")
                    nc.sync.dma_start(ct[:], r3(cumT[s]))
                    # broadcast delta row to 128 partitions via a K=1 matmul
                    dps = pp.tile([128, BC], F32, name=f"dps{s}", tag="ps")
                    mm_group(dps[:], [(ones_bf[0:1, 0:128],
                                       d1_sb[0:1, s * BC:(s + 1) * BC])])
                    d_bc = tp.tile([128, BC], F32, name=f"dbc{s}", tag="dbc",
                                   bufs=1)
                    nc.scalar.activation(d_bc[:], dps[:], AF.Copy)
                if s == 10:
                    nc.sync.dma_start(wo_sb[:], r3(woT))

                gates = [None] * KM

                def emit_pre(m):
                    sl = slice(128 * m, 128 * (m + 1))
                    ps = pp.tile([128, BC], F32, name=f"ps_{s}_{m}", tag="ps")
                    # PSUM pre-load: shared on the scalar engine, delta on the
                    # vector engine; matmuls then accumulate with start=False
                    nc.scalar.activation(ps[:], shared_sb[:, m, :], AF.Copy)
                    if not last:
                        nc.vector.scalar_tensor_tensor(
                            ps[:], d_bc[:], d_sb[:, m:m + 1], ps[:],
                            mybir.AluOpType.mult, mybir.AluOpType.add)
                    pre = []
                    src = ns_t if last else st
                    for t in (0, 1):
                        pre.append((a8_sb[:, 2 * t:2 * t + 2, sl],
                                    src[:, 2 * t:2 * t + 2, :], DR))
                    if not last:
                        for t in (0, 1):
                            pre.append((c8_sb[:, 2 * t:2 * t + 2, sl],
                                        ct[:, 2 * t:2 * t + 2, :], DR))
                    fin = []
                    if s > 0:
                        for t in (0, 1):
                            fin.append((wh8_sb[:, 2 * t:2 * t + 2, sl],
                                        h_rd[:, 2 * t:2 * t + 2, :], DR))
                    n_all = len(pre) + len(fin)
                    for i, term in enumerate(pre):
                        nc.tensor.matmul(ps[:], term[0], term[1],
                                         start=False, stop=(i == n_all - 1),
                                         perf_mode=term[2],
                                         skip_group_check=True)
                    return ps, fin, len(pre)

                def emit_fin(m, ps, fin, npre):
                    n_all = npre + len(fin)
                    for i, term in enumerate(fin):
                        nc.tensor.matmul(ps[:], term[0], term[1], start=False,
                                         stop=(npre + i == n_all - 1),
                                         perf_mode=term[2],
                                         skip_group_check=True)
                    gt = gp.tile([128, BC], F16, name=f"g_{s}_{m}", tag="gate")
                    nc.scalar.activation(gt[:], ps[:],
                                         MTANH if m // 4 == 2 else MSIG,
                                         scale=1.0 / WSCALE)
                    gates[m] = gt
                    # after chunk j completes (i,f,g,o present), update c/h
                    j = m - 12
                    if j >= 0:
                        ig, fg, gg, og = (gates[j], gates[4 + j], gates[8 + j],
                                          gates[12 + j])
                        tct = tp.tile([128, BC], F16, name=f"t_{s}_{j}", tag="tct")
                        if s == 0:
                            nc.vector.tensor_mul(c_t[j][:], ig[:], gg[:])
                        else:
                            t2 = tp.tile([128, BC], F32, name=f"u_{s}_{j}", tag="t2")
                            nc.vector.tensor_mul(t2[:], fg[:], c_t[j][:])
                            nc.vector.tensor_mul(c_t[j][:], ig[:], gg[:])
                            nc.vector.tensor_add(c_t[j][:], c_t[j][:], t2[:])
                        nc.scalar.activation(tct[:], c_t[j][:], MTANH)
                        nc.vector.tensor_mul(h_wr[:, j, :], og[:], tct[:])

                pend = []
                for m in M_ORDER:
                    pend.append((m, *emit_pre(m)))
                    if len(pend) > PIPE:
                        emit_fin(*pend.pop(0))
                while pend:
                    emit_fin(*pend.pop(0))

                if s in dbg:
                    cD, hD = dbg[s]
                    for j in range(KD):
                        nc.sync.dma_start(cD[128 * j:128 * (j + 1), :], c_t[j][:])
                        hcp = tp.tile([128, BC], F32, name=f"hcp{s}_{j}",
                                      tag="hcp")
                        nc.scalar.activation(hcp[:], h_wr[:, j, :], AF.Copy)
                        nc.sync.dma_start(hD[128 * j:128 * (j + 1), :], hcp[:])
                if not last:
                    # sims row for original slot s+1 (emitted at end of step:
                    # the ones-matmuls fill the PE while the h-tail resolves)
                    tsim = tp.tile([128, KD, BC], FP8, name=f"tm{s}", tag="tsim",
                                   bufs=1)
                    nc.vector.tensor_mul(tsim[:], st[:], ut[:])
                    sims_row(s + 1, tsim)
                if s == S - 2:
                    # g = sigmoid(max_s sims);  new_slot = g * P
                    nc.scalar.activation(g_row[:], max_row[:], MSIG)
                    gps = pp.tile([128, BC], F32, name="gps", tag="ps")
                    mm_group(gps[:], [(ones_bf[0:1, 0:128], g_row[:])])
                    G_bc = tp.tile([128, BC], F16, name="G_bc", tag="gbc", bufs=1)
                    nc.scalar.activation(G_bc[:], gps[:], AF.Copy)
                    ns_t = sp.tile([128, KD, BC], FP8, name="ns_t", tag="st")
                    for k in range(KD):
                        nc.vector.tensor_mul(ns_t[:, k, :], G_bc[:], P_t[:, k, :])

        # ================= epilogue =================
        with tc.tile_pool(name="ep", bufs=1) as ep:
            hp_sb = ep.tile([128, KD, BC], F16, name="hp_sb")
            nc.sync.dma_start(hp_sb[:], r3(hpT))
            x16 = ep.tile([128, KD, BC], F16, name="x16")
            nc.scalar.activation(x16[:], xt[:].bitcast(F32), AF.Copy)
            for m in range(KD):
                sl = slice(128 * m, 128 * (m + 1))
                eps = pp.tile([128, BC], F32, name=f"eps{m}", tag="ps")
                terms = [(wo_sb[:, j, sl], h15[:, j, :]) for j in range(KD)]
                terms += [(wo_sb[:, 4 + j, sl], hp_sb[:, j, :]) for j in range(KD)]
                terms += [(wo_sb[:, 8 + j, sl], x16[:, j, :]) for j in range(KD)]
                mm_group(eps[:], terms)
                out_t = ep.tile([128, BC], F32, name=f"o{m}", tag="out", bufs=2)
                nc.scalar.activation(out_t[:], eps[:], MTANH, bias=bo_sb[:, m:m + 1])
                nc.sync.dma_start(hnT[128 * m:128 * (m + 1), :], out_t[:])

        pp.release()
        st_p.release()
        wp.release()

    nc.compile()
    return nc


def kernel(**inputs):
    global _BUILT
    if _BUILT is None:
        _BUILT = _build_program()
    nc = _BUILT

    f32 = np.float32
    x = np.asarray(inputs["x_t"], f32)
    hp = np.asarray(inputs["h_prev"], f32)
    slots = np.asarray(inputs["slots"], f32)
    cum = np.asarray(inputs["cum_feats"], f32)
    dt = np.asarray(inputs["delta_t"], f32)
    Wk = np.asarray(inputs["Wk"], f32)
    Wq = np.asarray(inputs["Wq"], f32)
    Wv = np.asarray(inputs["Wv"], f32)
    bv = np.asarray(inputs["bv"], f32)
    Wr = np.asarray(inputs["Wr"], f32)
    br = np.asarray(inputs["br"], f32)
    Wa = np.asarray(inputs["Wa"], f32)
    ba = np.asarray(inputs["ba"], f32)
    W_ih = np.asarray(inputs["W_ih"], f32)
    W_hh = np.asarray(inputs["W_hh"], f32)
    b_ih = np.asarray(inputs["b_ih"], f32)
    b_hh = np.asarray(inputs["b_hh"], f32)
    Wo = np.asarray(inputs["Wo"], f32)
    bo = np.asarray(inputs["bo"], f32)

    xT = np.ascontiguousarray(x.T)
    hpT = hp.T.astype(NPF16)
    s0T = slots[:, 0, :].T.astype(NPF16)
    slotsT = slots[:, 1:, :].transpose(1, 2, 0).astype(NPF8)
    cumT = cum[:, 1:, :].transpose(1, 2, 0).astype(NPF8)
    d1T = np.ascontiguousarray((dt[:, 1:] + 1.0).T).astype(NPF16)

    m2T = np.ascontiguousarray(Wq.T @ Wk)
    wvT = np.ascontiguousarray(Wv.T)
    wrT = np.ascontiguousarray(Wr.T)
    waT = np.ascontiguousarray(Wa.T)
    A64 = WSCALE * W_ih[:, :D]
    C64 = WSCALE * W_ih[:, D:2 * D]
    aT = np.ascontiguousarray(A64.T).astype(NPF16)
    cT = np.ascontiguousarray(C64.T).astype(NPF16)
    a8 = np.ascontiguousarray(A64.T).astype(NPF8)
    c8 = np.ascontiguousarray(C64.T).astype(NPF8)
    wh8 = np.ascontiguousarray((WSCALE * W_hh).T).astype(NPF8)
    d_col = (WSCALE * W_ih[:, 2 * D]).reshape(G4, 1).astype(np.float32)
    bias_row = (WSCALE * (b_ih + b_hh)).reshape(1, G4).astype(NPF16)
    woT = Wo.T.astype(NPF16)

    shared_w = {
        "m2T": m2T, "wvT": wvT, "wrT": wrT, "waT": waT,
        "bv": bv.reshape(D, 1), "br": br.reshape(1, 1), "ba": ba.reshape(1, 1),
        "bo": bo.reshape(H, 1), "aT": aT, "cT": cT, "d_col": d_col,
        "bias_row": bias_row, "a8": a8, "c8": c8, "wh8": wh8, "woT": woT,
    }
    in_maps = []
    for c in range(NCORES):
        lo, hi = c * BC, (c + 1) * BC
        m = dict(shared_w)
        m["xT"] = xT[:, lo:hi]
        m["hpT"] = hpT[:, lo:hi]
        m["s0T"] = s0T[:, lo:hi]
        m["slotsT"] = slotsT[:, :, lo:hi]
        m["cumT"] = cumT[:, :, lo:hi]
        m["d1T"] = d1T[:, lo:hi].reshape(1, (S - 1) * BC)
        in_maps.append(m)

    res = bass_utils.run_bass_kernel_spmd(nc, in_maps, core_ids=list(range(NCORES)),
                                          **_RUN_KWARGS)
    global _LAST_RESULTS
    _LAST_RESULTS = res

    out = np.empty((B, H), np.float32)
    for c in range(NCORES):
        out[c * BC:(c + 1) * BC, :] = res.results[c]["hnT"].T
    return out


_RUN_KWARGS = {}
_LAST_RESULTS = None


# revision 15
# speedup vs baseline: 1.5094x; 1.0259x over previous
"""Trainium2 Bass kernel for nn_EventMemoryCell (B=4096, D=H=512, S=16).

Strategy (hardcoded for the spec shapes):
  - Data parallel over batch across 8 cores (512 rows each), parameters
    replicated; one SPMD NEFF.
  - Everything on-device runs in a transposed (feature-on-partition,
    batch-on-free) layout, so every matmul contracts over partitions and
    the LSTM recurrence needs no transposes.
  - mem_seq is never materialized: for s<15,
      xg[s] = A@slots_old[s+1] + C@cum_old[s+1] + d*(delta_old[s+1]+1)
              + shared,  shared = A@(leak*x) + 2*C@x + (b_ih+b_hh)
    and for s=15 xg[15] = A@new_slot + shared (cum part == 2x is in shared,
    delta part == 0).
  - Attention: sims = slots . ((Wk^T Wq) x), so keys (B,S,H) is never built.
  - The big per-step GEMMs (A/C over the slot/cum streams and W_hh over h)
    run in fp8-e4m3 with DoubleRow perf mode (2 fp8 k-rows per PE cell,
    2x throughput). Weights are pre-scaled x64 on the host so their 0.02-
    scale values clear the e4m3 denormal range; the gate activation applies
    scale=1/64 to undo it. The per-step "+ shared" lands in PSUM via an
    identity matmul and "+ d*delta" via a K=1 matmul, keeping the vector
    engine off the critical path.
  - Gate-tile emission is software-pipelined (stream-side matmuls of the
    next tiles are issued before the W_hh terms of earlier tiles) so the
    tensor engine doesn't stall on the h-recurrence tail.
"""
import sys

sys.path.insert(0, "/opt/trn_rl_repo")

import numpy as np
import ml_dtypes

import concourse.bass as bass
import concourse.tile as tile
import concourse.mybir as mybir
from concourse import bacc, bass_utils

F32 = mybir.dt.float32
F32R = mybir.dt.float32r
F16 = mybir.dt.float16
FP8 = mybir.dt.float8e4
AF = mybir.ActivationFunctionType
DR = mybir.MatmulPerfMode.DoubleRow
NPF16 = np.float16
NPF8 = ml_dtypes.float8_e4m3fn

B, S, D, H = 4096, 16, 512, 512
NCORES = 8
BC = B // NCORES            # 512 batch rows per core
G4 = 4 * H                  # 2048 gate rows
KD = D // 128               # 4 k-tiles over D/H
KM = G4 // 128              # 16 gate partition tiles
KO = (2 * H + D) // 128     # 12 k-tiles for the output projection
WSCALE = 64.0               # fp8 weight prescale (undone in gate activation)

# interleaved gate-tile order [0,4,8,12, 1,5,9,13, ...]: finish chunk j's
# i/f/g/o gates together so c/h updates start early
M_ORDER = [j + 4 * i for j in range(4) for i in range(4)]
PIPE = 5                    # gate-tile groups opened ahead of their Wh terms

_BUILT = None
DEBUG_STEPS = ()  # set before first kernel() call to dump c/h after these steps


def _build_program():
    nc = bacc.Bacc("TRN2", target_bir_lowering=False, debug=False)

    def din(name, shape, dt):
        return nc.dram_tensor(name, list(shape), dt, kind="ExternalInput").ap()

    xT = din("xT", (D, BC), F32R)
    s0T = din("s0T", (D, BC), F16)
    slotsT = din("slotsT", (S - 1, D, BC), FP8)
    cumT = din("cumT", (S - 1, D, BC), FP8)
    d1T = din("d1T", (1, (S - 1) * BC), F16)
    hpT = din("hpT", (H, BC), F16)
    m2T = din("m2T", (D, D), F32R)
    wvT = din("wvT", (D, D), F32R)
    wrT = din("wrT", (D, 1), F32R)
    waT = din("waT", (D, 1), F32R)
    bv = din("bv", (D, 1), F32)
    br = din("br", (1, 1), F32)
    ba = din("ba", (1, 1), F32)
    bo = din("bo", (H, 1), F32)
    aT = din("aT", (D, G4), F16)          # x64
    cT = din("cT", (D, G4), F16)          # x64
    bias_row = din("bias_row", (1, G4), F16)  # x64
    d_col = din("d_col", (G4, 1), F32)    # x64 delta column of W_ih
    a8 = din("a8", (D, G4), FP8)          # x64
    c8 = din("c8", (D, G4), FP8)          # x64
    wh8 = din("wh8", (H, G4), FP8)        # x64
    woT = din("woT", (2 * H + D, H), F16)
    hnT = nc.dram_tensor("hnT", [H, BC], F32, kind="ExternalOutput").ap()
    dbg = {}
    for ds in DEBUG_STEPS:
        dbg[ds] = (nc.dram_tensor(f"cD{ds}", [H, BC], F32, kind="ExternalOutput").ap(),
                   nc.dram_tensor(f"hD{ds}", [H, BC], F32, kind="ExternalOutput").ap())

    r3 = lambda ap: ap.rearrange("(kt p) b -> p kt b", p=128)
    r2 = lambda ap: ap.rearrange("(kt p) o -> p (kt o)", p=128)

    with tile.TileContext(nc) as tc:
        wp = tc.alloc_tile_pool(name="wp", bufs=1)
        st_p = tc.alloc_tile_pool(name="state", bufs=1)
        pp = tc.alloc_tile_pool(name="pp", bufs=8, space="PSUM")

        # ---- resident weights / constants ----
        a8_sb = wp.tile([128, KD, G4], FP8, name="a8_sb")
        c8_sb = wp.tile([128, KD, G4], FP8, name="c8_sb")
        wh8_sb = wp.tile([128, KD, G4], FP8, name="wh8_sb")
        d_sb = wp.tile([128, KM], F32, name="d_sb")
        nc.sync.dma_start(d_sb[:], r2(d_col))
        d1_sb = wp.tile([1, (S - 1) * BC], F16, name="d1_sb")
        nc.sync.dma_start(d1_sb[:], d1T)
        wo_sb = wp.tile([128, KO, H], F16, name="wo_sb")
        bo_sb = wp.tile([128, KD], F32, name="bo_sb")
        nc.sync.dma_start(bo_sb[:], r2(bo))
        ones_bf = wp.tile([1, BC], F16, name="ones_bf")
        nc.vector.memset(ones_bf[:], 1.0)
        ones8 = wp.tile([128, 2, 16], FP8, name="ones8")
        nc.vector.memset(ones8[:], 1.0)

        xt = st_p.tile([128, KD, BC], F32R, name="xt")
        nc.sync.dma_start(xt[:], r3(xT))
        shared_sb = st_p.tile([128, KM, BC], F16, name="shared_sb")
        ut = st_p.tile([128, KD, BC], FP8, name="ut")
        P_t = st_p.tile([128, KD, BC], F16, name="P_t")
        c_t = [st_p.tile([128, BC], F32, name=f"c{k}", tag=f"c{k}") for k in range(KD)]
        h8 = [st_p.tile([128, KD, BC], FP8, name=f"h8_{pq}", tag=f"h8_{pq}")
              for pq in range(2)]
        h15 = st_p.tile([128, KD, BC], F16, name="h15")
        g_row = st_p.tile([1, BC], F16, name="g_row")
        max_row = st_p.tile([1, BC], F32, name="max_row")

        MSIG, MTANH = AF.Sigmoid, AF.Tanh

        def sims_row(idx, tsrc):
            # running max over slots: max_row = max(max_row, slots_s . u)
            srp = pp.tile([128, BC], F32, name=f"srp{idx}", tag="ps")
            mm_group(srp[0:1, :], [(ones8[:, :, 0:1], tsrc[:, 2 * t:2 * t + 2, :], DR)
                                   for t in (0, 1)])
            if idx == 0:
                nc.scalar.activation(max_row[:], srp[0:1, :], AF.Copy)
            else:
                nc.vector.tensor_max(max_row[:], max_row[:], srp[0:1, :])

        def mm_group(ps_ap, terms):
            n = len(terms)
            for i, term in enumerate(terms):
                pm = term[2] if len(term) > 2 else None
                nc.tensor.matmul(ps_ap, term[0], term[1], start=(i == 0),
                                 stop=(i == n - 1), perf_mode=pm)

        # ================= prologue =================
        with tc.tile_pool(name="prop", bufs=1) as prop:
            m2_sb = prop.tile([128, KD, D], F32R, name="m2_sb")
            nc.sync.dma_start(m2_sb[:], r3(m2T))
            wv_sb = prop.tile([128, KD, D], F32R, name="wv_sb")
            nc.sync.dma_start(wv_sb[:], r3(wvT))
            wr_sb = prop.tile([128, KD], F32R, name="wr_sb")
            nc.sync.dma_start(wr_sb[:], r2(wrT))
            wa_sb = prop.tile([128, KD], F32R, name="wa_sb")
            nc.sync.dma_start(wa_sb[:], r2(waT))
            bv_sb = prop.tile([128, KD], F32, name="bv_sb")
            nc.sync.dma_start(bv_sb[:], r2(bv))
            br_sb = prop.tile([1, 1], F32, name="br_sb")
            nc.sync.dma_start(br_sb[:], br)
            ba_sb = prop.tile([1, 1], F32, name="ba_sb")
            nc.sync.dma_start(ba_sb[:], ba)
            s0_sb = prop.tile([128, KD, BC], F16, name="s0_sb")
            nc.sync.dma_start(s0_sb[:], r3(s0T))
            aT_sb = prop.tile([128, KD, G4], F16, name="aT_sb")
            nc.sync.dma_start(aT_sb[:], r3(aT))
            cT_sb = prop.tile([128, KD, G4], F16, name="cT_sb")
            nc.sync.dma_start(cT_sb[:], r3(cT))
            bias_sb = prop.tile([1, G4], F16, name="bias_sb")
            nc.sync.dma_start(bias_sb[:], bias_row)
            # big resident weights: emitted after the prologue-critical DMAs
            nc.sync.dma_start(a8_sb[:], r3(a8))
            nc.sync.dma_start(c8_sb[:], r3(c8))
            nc.sync.dma_start(wh8_sb[:], r3(wh8))
            vt = prop.tile([128, KD, BC], F16, name="vt")
            lx = prop.tile([128, KD, BC], F16, name="lx")
            xbf = prop.tile([128, KD, BC], F16, name="xbf")
            x2bf = prop.tile([128, KD, BC], F16, name="x2bf")
            r_row = prop.tile([1, BC], F16, name="r_row")
            lk_row = prop.tile([1, BC], F16, name="lk_row")
            R_bc = prop.tile([128, BC], F16, name="R_bc")
            L_bc = prop.tile([128, BC], F16, name="L_bc")

            nc.scalar.activation(xbf[:], xt[:].bitcast(F32), AF.Copy)
            nc.scalar.activation(x2bf[:], xt[:].bitcast(F32), AF.Copy, scale=2.0)

            # u = (Wk^T Wq) x ; v = Wv x + bv   (feature-major)
            for m in range(KD):
                ups = pp.tile([128, BC], F32, name=f"ups{m}", tag="ps")
                mm_group(ups[:], [(m2_sb[:, k, 128 * m:128 * (m + 1)], xt[:, k, :])
                                  for k in range(KD)])
                nc.scalar.activation(ut[:, m, :], ups[:], AF.Copy)
            for m in range(KD):
                vps = pp.tile([128, BC], F32, name=f"vps{m}", tag="ps")
                mm_group(vps[:], [(wv_sb[:, k, 128 * m:128 * (m + 1)], xt[:, k, :])
                                  for k in range(KD)])
                nc.scalar.activation(vt[:, m, :], vps[:], AF.Identity,
                                     bias=bv_sb[:, m:m + 1])

            # r / leak rows
            rps = pp.tile([128, BC], F32, name="rps", tag="ps")
            mm_group(rps[0:1, :], [(wr_sb[:, k:k + 1], xt[:, k, :]) for k in range(KD)])
            nc.scalar.activation(r_row[:], rps[0:1, :], MSIG, bias=br_sb[0:1, 0:1])
            lps = pp.tile([128, BC], F32, name="lps", tag="ps")
            mm_group(lps[0:1, :], [(wa_sb[:, k:k + 1], xt[:, k, :]) for k in range(KD)])
            nc.scalar.activation(lk_row[:], lps[0:1, :], MSIG, bias=ba_sb[0:1, 0:1])

            # broadcast r/leak rows to 128 partitions via a K=1 matmul
            bps = pp.tile([128, BC], F32, name="bps", tag="ps")
            mm_group(bps[:], [(ones_bf[0:1, 0:128], r_row[:])])
            nc.scalar.activation(R_bc[:], bps[:], AF.Copy)
            bps2 = pp.tile([128, BC], F32, name="bps2", tag="ps")
            mm_group(bps2[:], [(ones_bf[0:1, 0:128], lk_row[:])])
            nc.scalar.activation(L_bc[:], bps2[:], AF.Copy)

            # P = r*slots0 + (1-r)*v = v + r*(slots0 - v);  lx = leak*x
            for k in range(KD):
                t1 = prop.tile([128, BC], F16, name=f"pt{k}", tag="ptmp", bufs=2)
                nc.vector.tensor_sub(t1[:], s0_sb[:, k, :], vt[:, k, :])
                nc.vector.tensor_mul(t1[:], R_bc[:], t1[:])
                nc.vector.tensor_add(P_t[:, k, :], vt[:, k, :], t1[:])
                nc.vector.tensor_mul(lx[:, k, :], L_bc[:], xbf[:, k, :])

            # sims row 0 (original slot 0)
            ts0 = prop.tile([128, KD, BC], FP8, name="ts0")
            nc.vector.tensor_mul(ts0[:], s0_sb[:], ut[:])
            sims_row(0, ts0)

            # shared = A@lx + C@(2x) + (b_ih+b_hh) x ones   (all x64)
            for m in range(KM):
                sl = slice(128 * m, 128 * (m + 1))
                sps = pp.tile([128, BC], F32, name=f"sps{m}", tag="ps")
                terms = [(aT_sb[:, k, sl], lx[:, k, :]) for k in range(KD)]
                terms += [(cT_sb[:, k, sl], x2bf[:, k, :]) for k in range(KD)]
                terms += [(bias_sb[0:1, sl], ones_bf[:])]
                mm_group(sps[:], terms)
                nc.scalar.activation(shared_sb[:, m, :], sps[:], AF.Copy)

        # ================= LSTM over S steps =================
        with tc.tile_pool(name="sp", bufs=2) as sp, \
             tc.tile_pool(name="cp", bufs=2) as cp, \
             tc.tile_pool(name="gp", bufs=10) as gp, \
             tc.tile_pool(name="tp", bufs=2) as tp:
            ns_t = None
            for s in range(S):
                last = s == S - 1
                h_rd = h8[(s + 1) % 2]   # h[s-1]
                h_wr = h8[s % 2] if not last else h15
                if not last:
                    st = sp.tile([128, KD, BC], FP8, name=f"st{s}", tag="st")
                    nc.sync.dma_start(st[:], r3(slotsT[s]))
                    ct = cp.tile([128, KD, BC], FP8, name=f"ct{s}", tag="ct")
                    nc.sync.dma_start(ct[:], r3(cumT[s]))
                    # broadcast delta row to 128 partitions via a K=1 matmul
                    dps = pp.tile([128, BC], F32, name=f"dps{s}", tag="ps")
                    mm_group(dps[:], [(ones_bf[0:1, 0:128],
                                       d1_sb[0:1, s * BC:(s + 1) * BC])])
                    d_bc = tp.tile([128, BC], F32, name=f"dbc{s}", tag="dbc",
                                   bufs=1)
                    nc.scalar.activation(d_bc[:], dps[:], AF.Copy)
                if s == 10:
                    nc.sync.dma_start(wo_sb[:], r3(woT))

                gates = [None] * KM

                def emit_pre(m):
                    sl = slice(128 * m, 128 * (m + 1))
                    ps = pp.tile([128, BC], F32, name=f"ps_{s}_{m}", tag="ps")
                    # PSUM pre-load shared + d*delta in one vector op;
                    # matmuls then accumulate with start=False
                    if last:
                        nc.scalar.activation(ps[:], shared_sb[:, m, :], AF.Copy)
                    else:
                        nc.vector.scalar_tensor_tensor(
                            ps[:], d_bc[:], d_sb[:, m:m + 1], shared_sb[:, m, :],
                            mybir.AluOpType.mult, mybir.AluOpType.add)
                    pre = []
                    src = ns_t if last else st
                    for t in (0, 1):
                        pre.append((a8_sb[:, 2 * t:2 * t + 2, sl],
                                    src[:, 2 * t:2 * t + 2, :], DR))
                    if not last:
                        for t in (0, 1):
                            pre.append((c8_sb[:, 2 * t:2 * t + 2, sl],
                                        ct[:, 2 * t:2 * t + 2, :], DR))
                    fin = []
                    if s > 0:
                        for t in (0, 1):
                            fin.append((wh8_sb[:, 2 * t:2 * t + 2, sl],
                                        h_rd[:, 2 * t:2 * t + 2, :], DR))
                    n_all = len(pre) + len(fin)
                    for i, term in enumerate(pre):
                        nc.tensor.matmul(ps[:], term[0], term[1],
                                         start=False, stop=(i == n_all - 1),
                                         perf_mode=term[2],
                                         skip_group_check=True)
                    return ps, fin, len(pre)

                def emit_fin(m, ps, fin, npre):
                    n_all = npre + len(fin)
                    for i, term in enumerate(fin):
                        nc.tensor.matmul(ps[:], term[0], term[1], start=False,
                                         stop=(npre + i == n_all - 1),
                                         perf_mode=term[2],
                                         skip_group_check=True)
                    gt = gp.tile([128, BC], F16, name=f"g_{s}_{m}", tag="gate")
                    nc.scalar.activation(gt[:], ps[:],
                                         MTANH if m // 4 == 2 else MSIG,
                                         scale=1.0 / WSCALE)
                    gates[m] = gt
                    # after chunk j completes (i,f,g,o present), update c/h
                    j = m - 12
                    if j >= 0:
                        ig, fg, gg, og = (gates[j], gates[4 + j], gates[8 + j],
                                          gates[12 + j])
                        tct = tp.tile([128, BC], F16, name=f"t_{s}_{j}", tag="tct")
                        if s == 0:
                            nc.vector.tensor_mul(c_t[j][:], ig[:], gg[:])
                        else:
                            t2 = tp.tile([128, BC], F32, name=f"u_{s}_{j}", tag="t2")
                            nc.vector.tensor_mul(t2[:], fg[:], c_t[j][:])
                            nc.vector.tensor_mul(c_t[j][:], ig[:], gg[:])
                            nc.vector.tensor_add(c_t[j][:], c_t[j][:], t2[:])
                        nc.scalar.activation(tct[:], c_t[j][:], MTANH)
                        nc.vector.tensor_mul(h_wr[:, j, :], og[:], tct[:])

                pend = []
                for m in M_ORDER:
                    pend.append((m, *emit_pre(m)))
                    if len(pend) > PIPE:
                        emit_fin(*pend.pop(0))
                while pend:
                    emit_fin(*pend.pop(0))

                if s in dbg:
                    cD, hD = dbg[s]
                    for j in range(KD):
                        nc.sync.dma_start(cD[128 * j:128 * (j + 1), :], c_t[j][:])
                        hcp = tp.tile([128, BC], F32, name=f"hcp{s}_{j}",
                                      tag="hcp")
                        nc.scalar.activation(hcp[:], h_wr[:, j, :], AF.Copy)
                        nc.sync.dma_start(hD[128 * j:128 * (j + 1), :], hcp[:])
                if not last:
                    # sims row for original slot s+1 (emitted at end of step:
                    # the ones-matmuls fill the PE while the h-tail resolves)
                    tsim = tp.tile([128, KD, BC], FP8, name=f"tm{s}", tag="tsim",
                                   bufs=1)
                    nc.vector.tensor_mul(tsim[:], st[:], ut[:])
                    sims_row(s + 1, tsim)
                if s == S - 2:
                    # g = sigmoid(max_s sims);  new_slot = g * P
                    nc.scalar.activation(g_row[:], max_row[:], MSIG)
                    gps = pp.tile([128, BC], F32, name="gps", tag="ps")
                    mm_group(gps[:], [(ones_bf[0:1, 0:128], g_row[:])])
                    G_bc = tp.tile([128, BC], F16, name="G_bc", tag="gbc", bufs=1)
                    nc.scalar.activation(G_bc[:], gps[:], AF.Copy)
                    ns_t = sp.tile([128, KD, BC], FP8, name="ns_t", tag="st")
                    for k in range(KD):
                        nc.vector.tensor_mul(ns_t[:, k, :], G_bc[:], P_t[:, k, :])

        # ================= epilogue =================
        with tc.tile_pool(name="ep", bufs=1) as ep:
            hp_sb = ep.tile([128, KD, BC], F16, name="hp_sb")
            nc.sync.dma_start(hp_sb[:], r3(hpT))
            x16 = ep.tile([128, KD, BC], F16, name="x16")
            nc.scalar.activation(x16[:], xt[:].bitcast(F32), AF.Copy)
            for m in range(KD):
                sl = slice(128 * m, 128 * (m + 1))
                eps = pp.tile([128, BC], F32, name=f"eps{m}", tag="ps")
                terms = [(wo_sb[:, j, sl], h15[:, j, :]) for j in range(KD)]
                terms += [(wo_sb[:, 4 + j, sl], hp_sb[:, j, :]) for j in range(KD)]
                terms += [(wo_sb[:, 8 + j, sl], x16[:, j, :]) for j in range(KD)]
                mm_group(eps[:], terms)
                out_t = ep.tile([128, BC], F32, name=f"o{m}", tag="out", bufs=2)
                nc.scalar.activation(out_t[:], eps[:], MTANH, bias=bo_sb[:, m:m + 1])
                nc.sync.dma_start(hnT[128 * m:128 * (m + 1), :], out_t[:])

        pp.release()
        st_p.release()
        wp.release()

    nc.compile()
    return nc


def kernel(**inputs):
    global _BUILT
    if _BUILT is None:
        _BUILT = _build_program()
    nc = _BUILT

    f32 = np.float32
    x = np.asarray(inputs["x_t"], f32)
    hp = np.asarray(inputs["h_prev"], f32)
    slots = np.asarray(inputs["slots"], f32)
    cum = np.asarray(inputs["cum_feats"], f32)
    dt = np.asarray(inputs["delta_t"], f32)
    Wk = np.asarray(inputs["Wk"], f32)
    Wq = np.asarray(inputs["Wq"], f32)
    Wv = np.asarray(inputs["Wv"], f32)
    bv = np.asarray(inputs["bv"], f32)
    Wr = np.asarray(inputs["Wr"], f32)
    br = np.asarray(inputs["br"], f32)
    Wa = np.asarray(inputs["Wa"], f32)
    ba = np.asarray(inputs["ba"], f32)
    W_ih = np.asarray(inputs["W_ih"], f32)
    W_hh = np.asarray(inputs["W_hh"], f32)
    b_ih = np.asarray(inputs["b_ih"], f32)
    b_hh = np.asarray(inputs["b_hh"], f32)
    Wo = np.asarray(inputs["Wo"], f32)
    bo = np.asarray(inputs["bo"], f32)

    xT = np.ascontiguousarray(x.T)
    hpT = hp.T.astype(NPF16)
    s0T = slots[:, 0, :].T.astype(NPF16)
    slotsT = slots[:, 1:, :].transpose(1, 2, 0).astype(NPF8)
    cumT = cum[:, 1:, :].transpose(1, 2, 0).astype(NPF8)
    d1T = np.ascontiguousarray((dt[:, 1:] + 1.0).T).astype(NPF16)

    m2T = np.ascontiguousarray(Wq.T @ Wk)
    wvT = np.ascontiguousarray(Wv.T)
    wrT = np.ascontiguousarray(Wr.T)
    waT = np.ascontiguousarray(Wa.T)
    A64 = WSCALE * W_ih[:, :D]
    C64 = WSCALE * W_ih[:, D:2 * D]
    aT = np.ascontiguousarray(A64.T).astype(NPF16)
    cT = np.ascontiguousarray(C64.T).astype(NPF16)
    a8 = np.ascontiguousarray(A64.T).astype(NPF8)
    c8 = np.ascontiguousarray(C64.T).astype(NPF8)
    wh8 = np.ascontiguousarray((WSCALE * W_hh).T).astype(NPF8)
    d_col = (WSCALE * W_ih[:, 2 * D]).reshape(G4, 1).astype(np.float32)
    bias_row = (WSCALE * (b_ih + b_hh)).reshape(1, G4).astype(NPF16)
    woT = Wo.T.astype(NPF16)

    shared_w = {
        "m2T": m2T, "wvT": wvT, "wrT": wrT, "waT": waT,
        "bv": bv.reshape(D, 1), "br": br.reshape(1, 1), "ba": ba.reshape(1, 1),
        "bo": bo.reshape(H, 1), "aT": aT, "cT": cT, "d_col": d_col,
        "bias_row": bias_row, "a8": a8, "c8": c8, "wh8": wh8, "woT": woT,
    }
    in_maps = []
    for c in range(NCORES):
        lo, hi = c * BC, (c + 1) * BC
        m = dict(shared_w)
        m["xT"] = xT[:, lo:hi]
        m["hpT"] = hpT[:, lo:hi]
        m["s0T"] = s0T[:, lo:hi]
        m["slotsT"] = slotsT[:, :, lo:hi]
        m["cumT"] = cumT[:, :, lo:hi]
        m["d1T"] = d1T[:, lo:hi].reshape(1, (S - 1) * BC)
        in_maps.append(m)

    res = bass_utils.run_bass_kernel_spmd(nc, in_maps, core_ids=list(range(NCORES)),
                                          **_RUN_KWARGS)
    global _LAST_RESULTS
    _LAST_RESULTS = res

    out = np.empty((B, H), np.float32)
    for c in range(NCORES):
        out[c * BC:(c + 1) * BC, :] = res.results[c]["hnT"].T
    return out


_RUN_KWARGS = {}
_LAST_RESULTS = None


# revision 21
# speedup vs baseline: 1.5629x; 1.0354x over previous
"""Trainium2 Bass kernel for nn_EventMemoryCell (B=4096, D=H=512, S=16).

Strategy (hardcoded for the spec shapes):
  - Data parallel over batch across 8 cores (512 rows each), parameters
    replicated; one SPMD NEFF.
  - Everything on-device runs in a transposed (feature-on-partition,
    batch-on-free) layout, so every matmul contracts over partitions and
    the LSTM recurrence needs no transposes.
  - mem_seq is never materialized: for s<15,
      xg[s] = A@slots_old[s+1] + C@cum_old[s+1] + d*(delta_old[s+1]+1)
              + shared,  shared = A@(leak*x) + 2*C@x + (b_ih+b_hh)
    and for s=15 xg[15] = A@new_slot + shared (cum part == 2x is in shared,
    delta part == 0).
  - Attention: sims = slots . ((Wk^T Wq) x), so keys (B,S,H) is never built.
  - The big per-step GEMMs (A/C over the slot/cum streams and W_hh over h)
    run in fp8-e4m3 with DoubleRow perf mode (2 fp8 k-rows per PE cell,
    2x throughput). Weights are pre-scaled x64 on the host so their 0.02-
    scale values clear the e4m3 denormal range; the gate activation applies
    scale=1/64 to undo it. The per-step "+ shared" lands in PSUM via an
    identity matmul and "+ d*delta" via a K=1 matmul, keeping the vector
    engine off the critical path.
  - Gate-tile emission is software-pipelined (stream-side matmuls of the
    next tiles are issued before the W_hh terms of earlier tiles) so the
    tensor engine doesn't stall on the h-recurrence tail.
"""
import sys

sys.path.insert(0, "/opt/trn_rl_repo")

import numpy as np
import ml_dtypes

import concourse.bass as bass
import concourse.tile as tile
import concourse.mybir as mybir
from concourse import bacc, bass_utils

F32 = mybir.dt.float32
F32R = mybir.dt.float32r
F16 = mybir.dt.float16
FP8 = mybir.dt.float8e4
AF = mybir.ActivationFunctionType
DR = mybir.MatmulPerfMode.DoubleRow
NPF16 = np.float16
NPF8 = ml_dtypes.float8_e4m3fn

B, S, D, H = 4096, 16, 512, 512
NCORES = 8
BC = B // NCORES            # 512 batch rows per core
G4 = 4 * H                  # 2048 gate rows
KD = D // 128               # 4 k-tiles over D/H
KM = G4 // 128              # 16 gate partition tiles
KO = (2 * H + D) // 128     # 12 k-tiles for the output projection
WSCALE = 64.0               # fp8 weight prescale (undone in gate activation)

# interleaved gate-tile order [0,4,8,12, 1,5,9,13, ...]: finish chunk j's
# i/f/g/o gates together so c/h updates start early
M_ORDER = [j + 4 * i for j in range(4) for i in range(4)]
PIPE = 4                    # gate-tile groups opened ahead of their Wh terms

_BUILT = None
DEBUG_STEPS = ()  # set before first kernel() call to dump c/h after these steps


def _build_program():
    nc = bacc.Bacc("TRN2", target_bir_lowering=False, debug=False)

    def din(name, shape, dt):
        return nc.dram_tensor(name, list(shape), dt, kind="ExternalInput").ap()

    xT = din("xT", (D, BC), F32R)
    s0T = din("s0T", (D, BC), F16)
    slotsT = din("slotsT", (S - 1, D, BC), FP8)
    cumT = din("cumT", (S - 1, D, BC), FP8)
    d1T = din("d1T", (1, (S - 1) * BC), F16)
    hpT = din("hpT", (H, BC), F16)
    m2T = din("m2T", (D, D), F32R)
    wvT = din("wvT", (D, D), F32R)
    wrT = din("wrT", (D, 1), F32R)
    waT = din("waT", (D, 1), F32R)
    bv = din("bv", (D, 1), F32)
    br = din("br", (1, 1), F32)
    ba = din("ba", (1, 1), F32)
    bo = din("bo", (H, 1), F32)
    aT = din("aT", (D, G4), F16)          # x64
    cT = din("cT", (D, G4), F16)          # x64
    bias_row = din("bias_row", (1, G4), F16)  # x64
    d_col = din("d_col", (G4, 1), F32)    # x64 delta column of W_ih
    dr64 = din("dr64", (1, G4), F16)      # x64 delta column, row layout
    eyeT = din("eyeT", (128, 128), F16)
    a8 = din("a8", (D, G4), FP8)          # x64
    c8 = din("c8", (D, G4), FP8)          # x64
    wh8 = din("wh8", (H, G4), FP8)        # x64
    woT = din("woT", (2 * H + D, H), F16)
    hnT = nc.dram_tensor("hnT", [H, BC], F32, kind="ExternalOutput").ap()
    dbg = {}
    for ds in DEBUG_STEPS:
        dbg[ds] = (nc.dram_tensor(f"cD{ds}", [H, BC], F32, kind="ExternalOutput").ap(),
                   nc.dram_tensor(f"hD{ds}", [H, BC], F32, kind="ExternalOutput").ap())

    r3 = lambda ap: ap.rearrange("(kt p) b -> p kt b", p=128)
    r2 = lambda ap: ap.rearrange("(kt p) o -> p (kt o)", p=128)

    with tile.TileContext(nc) as tc:
        wp = tc.alloc_tile_pool(name="wp", bufs=1)
        st_p = tc.alloc_tile_pool(name="state", bufs=1)
        pp = tc.alloc_tile_pool(name="pp", bufs=8, space="PSUM")

        # ---- resident weights / constants ----
        a8_sb = wp.tile([128, KD, G4], FP8, name="a8_sb")
        c8_sb = wp.tile([128, KD, G4], FP8, name="c8_sb")
        wh8_sb = wp.tile([128, KD, G4], FP8, name="wh8_sb")
        d_sb = wp.tile([128, KM], F32, name="d_sb")
        nc.sync.dma_start(d_sb[:], r2(d_col))
        eye_sb = wp.tile([128, 128], F16, name="eye_sb")
        nc.sync.dma_start(eye_sb[:], eyeT)
        dr_sb = wp.tile([1, G4], F16, name="dr_sb")
        nc.sync.dma_start(dr_sb[:], dr64)
        d1_sb = wp.tile([1, (S - 1) * BC], F16, name="d1_sb")
        nc.sync.dma_start(d1_sb[:], d1T)
        wo_sb = wp.tile([128, KO, H], F16, name="wo_sb")
        bo_sb = wp.tile([128, KD], F32, name="bo_sb")
        nc.sync.dma_start(bo_sb[:], r2(bo))
        ones_bf = wp.tile([1, BC], F16, name="ones_bf")
        nc.vector.memset(ones_bf[:], 1.0)
        ones8 = wp.tile([128, 2, 16], FP8, name="ones8")
        nc.vector.memset(ones8[:], 1.0)

        xt = st_p.tile([128, KD, BC], F32R, name="xt")
        nc.sync.dma_start(xt[:], r3(xT))
        shared_sb = st_p.tile([128, KM, BC], F16, name="shared_sb")
        ut = st_p.tile([128, KD, BC], FP8, name="ut")
        P_t = st_p.tile([128, KD, BC], F16, name="P_t")
        c_t = [st_p.tile([128, BC], F16, name=f"c{k}", tag=f"c{k}") for k in range(KD)]
        h8 = [st_p.tile([128, KD, BC], FP8, name=f"h8_{pq}", tag=f"h8_{pq}")
              for pq in range(2)]
        h15 = st_p.tile([128, KD, BC], F16, name="h15")
        g_row = st_p.tile([1, BC], F16, name="g_row")
        max_row = st_p.tile([1, BC], F32, name="max_row")

        MSIG, MTANH = AF.Sigmoid, AF.Tanh

        def sims_row(idx, tsrc):
            # running max over slots: max_row = max(max_row, slots_s . u)
            srp = pp.tile([128, BC], F32, name=f"srp{idx}", tag="ps")
            mm_group(srp[0:1, :], [(ones8[:, :, 0:1], tsrc[:, 2 * t:2 * t + 2, :], DR)
                                   for t in (0, 1)])
            if idx == 0:
                nc.scalar.activation(max_row[:], srp[0:1, :], AF.Copy)
            else:
                nc.vector.tensor_max(max_row[:], max_row[:], srp[0:1, :])

        def mm_group(ps_ap, terms):
            n = len(terms)
            for i, term in enumerate(terms):
                pm = term[2] if len(term) > 2 else None
                nc.tensor.matmul(ps_ap, term[0], term[1], start=(i == 0),
                                 stop=(i == n - 1), perf_mode=pm)

        # ================= prologue =================
        with tc.tile_pool(name="prop", bufs=1) as prop:
            m2_sb = prop.tile([128, KD, D], F32R, name="m2_sb")
            nc.sync.dma_start(m2_sb[:], r3(m2T))
            wv_sb = prop.tile([128, KD, D], F32R, name="wv_sb")
            nc.sync.dma_start(wv_sb[:], r3(wvT))
            wr_sb = prop.tile([128, KD], F32R, name="wr_sb")
            nc.sync.dma_start(wr_sb[:], r2(wrT))
            wa_sb = prop.tile([128, KD], F32R, name="wa_sb")
            nc.sync.dma_start(wa_sb[:], r2(waT))
            bv_sb = prop.tile([128, KD], F32, name="bv_sb")
            nc.sync.dma_start(bv_sb[:], r2(bv))
            br_sb = prop.tile([1, 1], F32, name="br_sb")
            nc.sync.dma_start(br_sb[:], br)
            ba_sb = prop.tile([1, 1], F32, name="ba_sb")
            nc.sync.dma_start(ba_sb[:], ba)
            s0_sb = prop.tile([128, KD, BC], F16, name="s0_sb")
            nc.sync.dma_start(s0_sb[:], r3(s0T))
            aT_sb = prop.tile([128, KD, G4], F16, name="aT_sb")
            nc.sync.dma_start(aT_sb[:], r3(aT))
            cT_sb = prop.tile([128, KD, G4], F16, name="cT_sb")
            nc.sync.dma_start(cT_sb[:], r3(cT))
            bias_sb = prop.tile([1, G4], F16, name="bias_sb")
            nc.sync.dma_start(bias_sb[:], bias_row)
            # big resident weights: emitted after the prologue-critical DMAs
            nc.sync.dma_start(a8_sb[:], r3(a8))
            nc.sync.dma_start(c8_sb[:], r3(c8))
            nc.sync.dma_start(wh8_sb[:], r3(wh8))
            vt = prop.tile([128, KD, BC], F16, name="vt")
            lx = prop.tile([128, KD, BC], F16, name="lx")
            xbf = prop.tile([128, KD, BC], F16, name="xbf")
            x2bf = prop.tile([128, KD, BC], F16, name="x2bf")
            r_row = prop.tile([1, BC], F16, name="r_row")
            lk_row = prop.tile([1, BC], F16, name="lk_row")
            R_bc = prop.tile([128, BC], F16, name="R_bc")
            L_bc = prop.tile([128, BC], F16, name="L_bc")

            nc.scalar.activation(xbf[:], xt[:].bitcast(F32), AF.Copy)
            nc.scalar.activation(x2bf[:], xt[:].bitcast(F32), AF.Copy, scale=2.0)

            # u = (Wk^T Wq) x ; v = Wv x + bv   (feature-major)
            for m in range(KD):
                ups = pp.tile([128, BC], F32, name=f"ups{m}", tag="ps")
                mm_group(ups[:], [(m2_sb[:, k, 128 * m:128 * (m + 1)], xt[:, k, :])
                                  for k in range(KD)])
                nc.scalar.activation(ut[:, m, :], ups[:], AF.Copy)
            for m in range(KD):
                vps = pp.tile([128, BC], F32, name=f"vps{m}", tag="ps")
                mm_group(vps[:], [(wv_sb[:, k, 128 * m:128 * (m + 1)], xt[:, k, :])
                                  for k in range(KD)])
                nc.scalar.activation(vt[:, m, :], vps[:], AF.Identity,
                                     bias=bv_sb[:, m:m + 1])

            # r / leak rows
            rps = pp.tile([128, BC], F32, name="rps", tag="ps")
            mm_group(rps[0:1, :], [(wr_sb[:, k:k + 1], xt[:, k, :]) for k in range(KD)])
            nc.scalar.activation(r_row[:], rps[0:1, :], MSIG, bias=br_sb[0:1, 0:1])
            lps = pp.tile([128, BC], F32, name="lps", tag="ps")
            mm_group(lps[0:1, :], [(wa_sb[:, k:k + 1], xt[:, k, :]) for k in range(KD)])
            nc.scalar.activation(lk_row[:], lps[0:1, :], MSIG, bias=ba_sb[0:1, 0:1])

            # broadcast r/leak rows to 128 partitions via a K=1 matmul
            bps = pp.tile([128, BC], F32, name="bps", tag="ps")
            mm_group(bps[:], [(ones_bf[0:1, 0:128], r_row[:])])
            nc.scalar.activation(R_bc[:], bps[:], AF.Copy)
            bps2 = pp.tile([128, BC], F32, name="bps2", tag="ps")
            mm_group(bps2[:], [(ones_bf[0:1, 0:128], lk_row[:])])
            nc.scalar.activation(L_bc[:], bps2[:], AF.Copy)

            # P = r*slots0 + (1-r)*v = v + r*(slots0 - v);  lx = leak*x
            for k in range(KD):
                t1 = prop.tile([128, BC], F16, name=f"pt{k}", tag="ptmp", bufs=2)
                nc.vector.tensor_sub(t1[:], s0_sb[:, k, :], vt[:, k, :])
                nc.vector.tensor_mul(t1[:], R_bc[:], t1[:])
                nc.vector.tensor_add(P_t[:, k, :], vt[:, k, :], t1[:])
                nc.vector.tensor_mul(lx[:, k, :], L_bc[:], xbf[:, k, :])

            # sims row 0 (original slot 0)
            ts0 = prop.tile([128, KD, BC], FP8, name="ts0")
            nc.vector.tensor_mul(ts0[:], s0_sb[:], ut[:])
            sims_row(0, ts0)

            # shared = A@lx + C@(2x) + (b_ih+b_hh) x ones   (all x64)
            for m in range(KM):
                sl = slice(128 * m, 128 * (m + 1))
                sps = pp.tile([128, BC], F32, name=f"sps{m}", tag="ps")
                terms = [(aT_sb[:, k, sl], lx[:, k, :]) for k in range(KD)]
                terms += [(cT_sb[:, k, sl], x2bf[:, k, :]) for k in range(KD)]
                terms += [(bias_sb[0:1, sl], ones_bf[:])]
                mm_group(sps[:], terms)
                nc.scalar.activation(shared_sb[:, m, :], sps[:], AF.Copy)

        # ================= LSTM over S steps =================
        # One global software pipeline over all (step, gate-tile) items. The
        # first TPRE tiles of each step pre-load shared+delta via tensor-
        # engine matmuls (identity + K=1) so the PE crosses the step
        # boundary without waiting on the vector engine; the rest pre-load
        # via a single vector STT (shared + d*delta in one op).
        TPRE = 4
        with tc.tile_pool(name="sp", bufs=2) as sp, \
             tc.tile_pool(name="cp", bufs=2) as cp, \
             tc.tile_pool(name="gp", bufs=10) as gp, \
             tc.tile_pool(name="tp", bufs=2) as tp:
            st_t = [None] * S
            ct_t = [None] * S
            dbc_t = [None] * S
            gates_t = {}
            ns_holder = [None]
            fin_count = [0] * S

            def prefetch(s):
                if s > S - 2:
                    return
                st = sp.tile([128, KD, BC], FP8, name=f"st{s}", tag="st")
                nc.sync.dma_start(st[:], r3(slotsT[s]))
                ct = cp.tile([128, KD, BC], FP8, name=f"ct{s}", tag="ct")
                nc.sync.dma_start(ct[:], r3(cumT[s]))
                # broadcast delta row to 128 partitions via a K=1 matmul
                dps = pp.tile([128, BC], F32, name=f"dps{s}", tag="ps")
                mm_group(dps[:], [(ones_bf[0:1, 0:128],
                                   d1_sb[0:1, s * BC:(s + 1) * BC])])
                dbc = tp.tile([128, BC], F32, name=f"dbc{s}", tag="dbc", bufs=2)
                nc.scalar.activation(dbc[:], dps[:], AF.Copy)
                st_t[s], ct_t[s], dbc_t[s] = st, ct, dbc

            def emit_tsim(s):
                # sims row for original slot s+1 from step-s stream tile
                tsim = tp.tile([128, KD, BC], FP8, name=f"tm{s}", tag="tsim",
                               bufs=2)
                nc.vector.tensor_mul(tsim[:], st_t[s][:], ut[:])
                sims_row(s + 1, tsim)

            def emit_ns():
                # g = sigmoid(max_s sims);  new_slot = g * P
                nc.scalar.activation(g_row[:], max_row[:], MSIG)
                gps = pp.tile([128, BC], F32, name="gps", tag="ps")
                mm_group(gps[:], [(ones_bf[0:1, 0:128], g_row[:])])
                G_bc = tp.tile([128, BC], F16, name="G_bc", tag="gbc", bufs=1)
                nc.scalar.activation(G_bc[:], gps[:], AF.Copy)
                ns_t = sp.tile([128, KD, BC], FP8, name="ns_t", tag="st")
                for k in range(KD):
                    nc.vector.tensor_mul(ns_t[:, k, :], G_bc[:], P_t[:, k, :])
                ns_holder[0] = ns_t

            def emit_pre(s, m, idx):
                last = s == S - 1
                sl = slice(128 * m, 128 * (m + 1))
                ps = pp.tile([128, BC], F32, name=f"ps_{s}_{m}", tag="ps")
                tensor_pre = idx < TPRE
                pre = []
                if tensor_pre:
                    pre.append((eye_sb[:], shared_sb[:, m, :], None))
                    if not last:
                        pre.append((dr_sb[0:1, sl],
                                    d1_sb[0:1, s * BC:(s + 1) * BC], None))
                elif last:
                    nc.scalar.activation(ps[:], shared_sb[:, m, :], AF.Copy)
                else:
                    nc.vector.scalar_tensor_tensor(
                        ps[:], dbc_t[s][:], d_sb[:, m:m + 1],
                        shared_sb[:, m, :],
                        mybir.AluOpType.mult, mybir.AluOpType.add)
                src = ns_holder[0] if last else st_t[s]
                for t in (0, 1):
                    pre.append((a8_sb[:, 2 * t:2 * t + 2, sl],
                                src[:, 2 * t:2 * t + 2, :], DR))
                if not last:
                    for t in (0, 1):
                        pre.append((c8_sb[:, 2 * t:2 * t + 2, sl],
                                    ct_t[s][:, 2 * t:2 * t + 2, :], DR))
                fin = []
                if s > 0:
                    for t in (0, 1):
                        fin.append((wh8_sb[:, 2 * t:2 * t + 2, sl],
                                    h8[(s + 1) % 2][:, 2 * t:2 * t + 2, :], DR))
                n_all = len(pre) + len(fin)
                for i, term in enumerate(pre):
                    nc.tensor.matmul(ps[:], term[0], term[1],
                                     start=(tensor_pre and i == 0),
                                     stop=(i == n_all - 1),
                                     perf_mode=term[2],
                                     skip_group_check=not tensor_pre)
                return ps, fin, len(pre)

            def emit_fin(s, m, ps, fin, npre):
                last = s == S - 1
                gates = gates_t[s]
                h_wr = h15 if last else h8[s % 2]
                n_all = npre + len(fin)
                for i, term in enumerate(fin):
                    nc.tensor.matmul(ps[:], term[0], term[1], start=False,
                                     stop=(npre + i == n_all - 1),
                                     perf_mode=term[2],
                                     skip_group_check=True)
                gt = gp.tile([128, BC], F16, name=f"g_{s}_{m}", tag="gate")
                nc.scalar.activation(gt[:], ps[:],
                                     MTANH if m // 4 == 2 else MSIG,
                                     scale=1.0 / WSCALE)
                gates[m] = gt
                # after chunk j completes (i,f,g,o present), update c/h
                j = m - 12
                if j >= 0:
                    ig, fg, gg, og = (gates[j], gates[4 + j], gates[8 + j],
                                      gates[12 + j])
                    tct = tp.tile([128, BC], F16, name=f"t_{s}_{j}", tag="tct")
                    if s == 0:
                        nc.vector.tensor_mul(c_t[j][:], ig[:], gg[:])
                    else:
                        t2 = tp.tile([128, BC], F16, name=f"u_{s}_{j}", tag="t2")
                        nc.vector.tensor_mul(t2[:], fg[:], c_t[j][:])
                        nc.vector.tensor_mul(c_t[j][:], ig[:], gg[:])
                        nc.vector.tensor_add(c_t[j][:], c_t[j][:], t2[:])
                    nc.scalar.activation(tct[:], c_t[j][:], MTANH)
                    nc.vector.tensor_mul(h_wr[:, j, :], og[:], tct[:])
                fin_count[s] += 1
                if fin_count[s] == KM and s in dbg:
                    cD, hD = dbg[s]
                    for jj in range(KD):
                        nc.sync.dma_start(cD[128 * jj:128 * (jj + 1), :],
                                          c_t[jj][:])
                        hcp = tp.tile([128, BC], F32, name=f"hcp{s}_{jj}",
                                      tag="hcp")
                        nc.scalar.activation(hcp[:], h_wr[:, jj, :], AF.Copy)
                        nc.sync.dma_start(hD[128 * jj:128 * (jj + 1), :],
                                          hcp[:])

            prefetch(0)
            pend = []
            for s in range(S):
                gates_t[s] = [None] * KM
                if s == 10:
                    nc.sync.dma_start(wo_sb[:], r3(woT))
                if s == S - 2:
                    emit_tsim(s)
                    emit_ns()
                for idx, m in enumerate(M_ORDER):
                    if idx == 4:
                        prefetch(s + 1)
                    if idx == 8 and s < S - 2:
                        emit_tsim(s)
                    pend.append((s, m, *emit_pre(s, m, idx)))
                    if len(pend) > PIPE:
                        emit_fin(*pend.pop(0))
            while pend:
                emit_fin(*pend.pop(0))

        # ================= epilogue =================
        with tc.tile_pool(name="ep", bufs=1) as ep:
            hp_sb = ep.tile([128, KD, BC], F16, name="hp_sb")
            nc.sync.dma_start(hp_sb[:], r3(hpT))
            x16 = ep.tile([128, KD, BC], F16, name="x16")
            nc.scalar.activation(x16[:], xt[:].bitcast(F32), AF.Copy)
            for m in range(KD):
                sl = slice(128 * m, 128 * (m + 1))
                eps = pp.tile([128, BC], F32, name=f"eps{m}", tag="ps")
                terms = [(wo_sb[:, j, sl], h15[:, j, :]) for j in range(KD)]
                terms += [(wo_sb[:, 4 + j, sl], hp_sb[:, j, :]) for j in range(KD)]
                terms += [(wo_sb[:, 8 + j, sl], x16[:, j, :]) for j in range(KD)]
                mm_group(eps[:], terms)
                out_t = ep.tile([128, BC], F32, name=f"o{m}", tag="out", bufs=2)
                nc.scalar.activation(out_t[:], eps[:], MTANH, bias=bo_sb[:, m:m + 1])
                nc.sync.dma_start(hnT[128 * m:128 * (m + 1), :], out_t[:])

        pp.release()
        st_p.release()
        wp.release()

    nc.compile()
    return nc


def kernel(**inputs):
    global _BUILT
    if _BUILT is None:
        _BUILT = _build_program()
    nc = _BUILT

    f32 = np.float32
    x = np.asarray(inputs["x_t"], f32)
    hp = np.asarray(inputs["h_prev"], f32)
    slots = np.asarray(inputs["slots"], f32)
    cum = np.asarray(inputs["cum_feats"], f32)
    dt = np.asarray(inputs["delta_t"], f32)
    Wk = np.asarray(inputs["Wk"], f32)
    Wq = np.asarray(inputs["Wq"], f32)
    Wv = np.asarray(inputs["Wv"], f32)
    bv = np.asarray(inputs["bv"], f32)
    Wr = np.asarray(inputs["Wr"], f32)
    br = np.asarray(inputs["br"], f32)
    Wa = np.asarray(inputs["Wa"], f32)
    ba = np.asarray(inputs["ba"], f32)
    W_ih = np.asarray(inputs["W_ih"], f32)
    W_hh = np.asarray(inputs["W_hh"], f32)
    b_ih = np.asarray(inputs["b_ih"], f32)
    b_hh = np.asarray(inputs["b_hh"], f32)
    Wo = np.asarray(inputs["Wo"], f32)
    bo = np.asarray(inputs["bo"], f32)

    xT = np.ascontiguousarray(x.T)
    hpT = hp.T.astype(NPF16)
    s0T = slots[:, 0, :].T.astype(NPF16)
    slotsT = slots[:, 1:, :].transpose(1, 2, 0).astype(NPF8)
    cumT = cum[:, 1:, :].transpose(1, 2, 0).astype(NPF8)
    d1T = np.ascontiguousarray((dt[:, 1:] + 1.0).T).astype(NPF16)

    m2T = np.ascontiguousarray(Wq.T @ Wk)
    wvT = np.ascontiguousarray(Wv.T)
    wrT = np.ascontiguousarray(Wr.T)
    waT = np.ascontiguousarray(Wa.T)
    A64 = WSCALE * W_ih[:, :D]
    C64 = WSCALE * W_ih[:, D:2 * D]
    aT = np.ascontiguousarray(A64.T).astype(NPF16)
    cT = np.ascontiguousarray(C64.T).astype(NPF16)
    a8 = np.ascontiguousarray(A64.T).astype(NPF8)
    c8 = np.ascontiguousarray(C64.T).astype(NPF8)
    wh8 = np.ascontiguousarray((WSCALE * W_hh).T).astype(NPF8)
    d_col = (WSCALE * W_ih[:, 2 * D]).reshape(G4, 1).astype(np.float32)
    dr64 = (WSCALE * W_ih[:, 2 * D]).reshape(1, G4).astype(NPF16)
    bias_row = (WSCALE * (b_ih + b_hh)).reshape(1, G4).astype(NPF16)
    woT = Wo.T.astype(NPF16)
    eye = np.eye(128, dtype=NPF16)

    shared_w = {
        "m2T": m2T, "wvT": wvT, "wrT": wrT, "waT": waT,
        "bv": bv.reshape(D, 1), "br": br.reshape(1, 1), "ba": ba.reshape(1, 1),
        "bo": bo.reshape(H, 1), "aT": aT, "cT": cT, "d_col": d_col,
        "dr64": dr64, "bias_row": bias_row, "a8": a8, "c8": c8, "wh8": wh8,
        "woT": woT, "eyeT": eye,
    }
    in_maps = []
    for c in range(NCORES):
        lo, hi = c * BC, (c + 1) * BC
        m = dict(shared_w)
        m["xT"] = xT[:, lo:hi]
        m["hpT"] = hpT[:, lo:hi]
        m["s0T"] = s0T[:, lo:hi]
        m["slotsT"] = slotsT[:, :, lo:hi]
        m["cumT"] = cumT[:, :, lo:hi]
        m["d1T"] = d1T[:, lo:hi].reshape(1, (S - 1) * BC)
        in_maps.append(m)

    res = bass_utils.run_bass_kernel_spmd(nc, in_maps, core_ids=list(range(NCORES)),
                                          **_RUN_KWARGS)
    global _LAST_RESULTS
    _LAST_RESULTS = res

    out = np.empty((B, H), np.float32)
    for c in range(NCORES):
        out[c * BC:(c + 1) * BC, :] = res.results[c]["hnT"].T
    return out


_RUN_KWARGS = {}
_LAST_RESULTS = None
